# revision 46
# baseline (speedup 1.0000x reference)
"""Trainium2 Bass kernel for the autoregressive GRU decode head.

Problem: context = mean over zones of encoded_features[:, -1]  -> (B, D)
then 12 autoregressive steps of a 2-layer GRU (H=256) + linear projection
to N=256 zones.  B=1024, data-parallel across 8 NeuronCores (128 batch each).

Structure (per core, feature-major / "transposed" activations):
  actT (128p, 4 slots, 128) bf16 : [h0 c0, h0 c1, h1 c0, h1 c1]
       slot holds h[b, c*128 + p] at [p, b]    (c = chunk of the 256-dim)
  Gate tensors (PSUM) use layout [p, c*128 + b].
  Matmuls: out(gate_chunk, B) = lhsT.T @ rhs, lhsT = W^T tile (K<=128, M=128),
  rhs = actT slot (K=128, B=128), K-chunks accumulated in PSUM.
  The prediction feedback is algebraically folded into layer 0's weights:
  W_pred @ (W_out h1 + b_out) = (W_pred W_out) h1 + W_pred b_out.
  Chain per layer (PSUM g_rz = [r|z], g_hi = [ghn|gin]):
    r,z = sigmoid(g_rz + bias)   4x 128-wide ACT (bias via ACT operand)
    t = (ghn + bhn) * r          STT 256w
    v = (gin + bin) + t          STT 256w
    n = tanh(v); c = z*h; q = (z-1)*n; h' = c - q
  Critical-path trick: the NEXT matmul block needs W @ h' with h' = c - q,
  so it is issued as W @ c + (-W) @ q against the c/q tiles, which exist
  ~1us before h' does -- the r-gate matmuls of the next layer/step complete
  almost immediately after the chain, instead of serializing behind h'.
  The z / in-gate matmuls use h' directly (they are needed later).
  All h1(t-1)-only work runs during chain0; all h0'(t)-only work (incl.
  step t+1's rz0/hn0 h-parts) during chain1; wout(t-1) during chain0.
The encoded_features slice is streamed as bf16 (host-converted) in 6 chunks
(64,64,64,32,16,16 zones) ALL on the sync HWDGE queue so arrival order
matches the tree's processing order; weights/biases ride the scalar queue.
The zone mean is a DVE-only pairwise TT-add tree (bf16 upper levels, f32
tail) overlapped with the DMA.  GpSimd is avoided: it shares an SBUF port
with the DVE and large concurrent ops slow both ~4x.
"""

import sys

for _p in ("/opt/trn_rl_repo",):
    if _p not in sys.path:
        sys.path.insert(0, _p)

import numpy as np
import ml_dtypes

import concourse.bass as bass
import concourse.tile as tile
from concourse import mybir
from concourse.vector_clock import ScopedClock

BF16 = ml_dtypes.bfloat16

B, T, NZ, D = 1024, 8, 256, 256
H = 256
STEPS = 12
N_CORES = 8
PC = B // N_CORES  # 128 batch per core

F32 = mybir.dt.float32
BF = mybir.dt.bfloat16
F8 = mybir.dt.float8e4
AF = mybir.ActivationFunctionType
OP = mybir.AluOpType

# biasT (bf16, [8, NGRP*128]): per-group bias stacks for the K=8 bias
# matmuls (one per PSUM bank+step: a start=True matmul resets the WHOLE
# bank, so each bank gets exactly one group whose first matmul adds all
# four 128-chunk biases).  Group g's lhsT is biasT[0:8, g*128:(g+1)*128];
# rows 0:4 hold bf16-hi chunk biases, rows 4:8 the bf16-lo residuals.
BT_RZ0 = 0                    # 12 steps: [r c0, r c1, z c0, z c1] of L0
BT_RZ1 = BT_RZ0 + STEPS
BT_HI0 = BT_RZ1 + 1           # 12 steps: [hn c0, hn c1, in c0, in c1]
BT_HI1 = BT_HI0 + STEPS
NGRP = BT_HI1 + 1

# brow (bf16, [1, 2*128]): b_out as a moving row for the batch-major wout
_BOUT = 0
NROW = 2


def _install_tile_drain_patch():
    """walrus (CoreV3) rejects >1 sync wait on the tail drain; spill extras
    onto preceding sync nops."""
    if getattr(tile.TileContext, "_drain_patch_installed", False):
        return

    def _patched(self, tick_clock, wait_clock):
        nc = self.nc
        bb = nc.cur_bb.bb
        drain_bi = nc.sync.drain()
        drain_inst = drain_bi.ins
        wait_clock.add_sem_waits(
            drain_inst, ScopedClock({None: tick_clock.global_clock})
        )
        w = drain_inst.sync_info.on_wait if drain_inst.sync_info else None
        maxw = 1
        if w and len(w) > maxw:
            extra = list(w[maxw:])
            drain_inst.sync_info.on_wait = list(w[:maxw])
            idx = bb.instructions.index(drain_inst)
            nops = []
            for i in range(0, len(extra), maxw):
                nop_bi = nc.sync.nop()
                nop = nop_bi.ins
                si = nop.sync_info
                nop.sync_info = mybir.SyncInfo(
                    on_wait=extra[i : i + maxw],
                    on_update=(si.on_update if si else []),
                )
                bb.instructions.remove(nop)
                nops.append(nop)
            bb.instructions[idx:idx] = nops
        nc.all_engine_barrier()
        popped = nc._tile_sem_poison_stack.pop()
        assert popped is self._sem_poison
        nc.clear_and_free_semaphores(list(self.sems.allocated().values()))
        nc.all_engine_barrier()

    tile.TileContext._drain_and_barrier = _patched
    tile.TileContext._drain_patch_installed = True


def _split_waits(nc, maxw=1):
    """This walrus build rejects instructions carrying more than ~1 sem
    wait; spill extra waits onto same-engine nops placed just before."""
    for bb in nc.main_func.blocks:
        new_list = []
        for inst in bb.instructions:
            si = inst.sync_info
            w = list(si.on_wait) if si and si.on_wait else []
            if len(w) > maxw:
                keep = w[len(w) - maxw:]
                extra = w[: len(w) - maxw]
                si.on_wait = keep
                for i in range(0, len(extra), maxw):
                    nop = mybir.InstNoOp(
                        name=f"{inst.name}-sw{i}", ins=[], outs=[]
                    )
                    nop.engine = inst.engine
                    nop.sync_info = mybir.SyncInfo(
                        on_wait=extra[i : i + maxw], on_update=[]
                    )
                    nc.register_instruction(nop)
                    new_list.append(nop)
            new_list.append(inst)
        bb.instructions[:] = new_list


class _Group:
    """Tracks start/stop flags for a PSUM accumulation group whose matmuls
    are emitted in several program-order batches."""

    def __init__(self, total):
        self.total = total
        self.emitted = 0

    def flags(self):
        start = self.emitted == 0
        self.emitted += 1
        return start, self.emitted == self.total


FILL_CQ = 0     # junk matmuls before each layer.s c/q-dependent block
FILL_P1 = 0      # before each phase-1 PE zone-sum chunk group
FILL_RAMP = 0   # bridging the phase-1 -> decode transition


def build_kernel(nsteps=12):
    """Build the per-core Bass graph (SPMD: same graph on all 8 cores)."""
    _install_tile_drain_patch()
    nc = bass.Bass()

    enc = nc.declare_dram_parameter("enc", [PC, NZ, D], F8, isOutput=False)
    wrz0 = nc.declare_dram_parameter("wrz0", [4, 128, 512], BF, isOutput=False)
    win0 = nc.declare_dram_parameter("win0", [2, 128, 256], BF, isOutput=False)
    whn0 = nc.declare_dram_parameter("whn0", [2, 128, 256], BF, isOutput=False)
    wrz1 = nc.declare_dram_parameter("wrz1", [4, 128, 512], BF, isOutput=False)
    win1 = nc.declare_dram_parameter("win1", [2, 128, 256], BF, isOutput=False)
    whn1 = nc.declare_dram_parameter("whn1", [2, 128, 256], BF, isOutput=False)
    wout = nc.declare_dram_parameter("wout", [2, 128, 256], BF, isOutput=False)
    biasT = nc.declare_dram_parameter("biasT", [8, NGRP * 128], BF,
                                      isOutput=False)
    oh8 = nc.declare_dram_parameter("oh8", [8, 512], BF, isOutput=False)
    brows = nc.declare_dram_parameter("brows", [1, NROW * 128], BF,
                                      isOutput=False)
    out = nc.declare_dram_parameter("out", [PC, STEPS, NZ], BF, isOutput=True)

    with tile.TileContext(nc) as tc:
        with (
            tc.tile_pool(name="consts", bufs=1) as consts,
            tc.tile_pool(name="state", bufs=1) as state,
            tc.tile_pool(name="enc_pool", bufs=4) as enc_pool,
            tc.tile_pool(name="gates", bufs=2) as gates,
            tc.tile_pool(name="ostage", bufs=2) as ostage,
            tc.tile_pool(name="psum", bufs=1, space="PSUM") as psum,
        ):
            # ---- phase 1 DMA: enc all on the sync queue, in tree order ----
            # PE consumes ~2x faster than the DVE tree: it owns the first
            # chunks plus the tail; the DVE takes two early-arriving chunks
            ZCHS = [32, 32, 32, 32, 32, 32, 32, 16, 16]
            PE_CHUNKS = (0, 2, 4, 5, 6, 7, 8)
            NCH = len(ZCHS)
            # enc split across both HWDGE queues (even->sync, odd->scalar)
            # so the two queues' per-chunk completion overheads overlap and
            # the aggregate stream runs at the DMA bandwidth roofline
            e_tiles = []
            z0 = 0
            for i, zch in enumerate(ZCHS):
                e_sb = enc_pool.tile([128, 32 * D], F8, tag="echunk",
                                     bufs=13)
                eng = nc.sync if i % 2 == 0 else nc.scalar
                eng.dma_start(e_sb[:, : zch * D], enc[:, z0 : z0 + zch, :])
                e_tiles.append(e_sb)
                z0 += zch

            # small constants then weights behind the enc stream, ordered
            # by first use
            biasT_sb = consts.tile([8, NGRP * 128], BF, tag="biasT")
            nc.scalar.dma_start(biasT_sb[:], biasT[:])
            oh8_sb = consts.tile([8, 512], BF, tag="oh8")
            nc.scalar.dma_start(oh8_sb[:], oh8[:])
            brow_sb = consts.tile([1, NROW * 128], BF, tag="brow")
            nc.scalar.dma_start(brow_sb[:], brows[:])
            w_sb = {}
            for name, ap, kc, mdim in (
                ("wrz0", wrz0, 4, 512),
                ("whn0", whn0, 2, 256),
                ("wrz1", wrz1, 4, 512),
                ("whn1", whn1, 2, 256),
                ("win1", win1, 2, 256),
                ("wout", wout, 2, 256),
                ("win0", win0, 2, 256),
            ):
                t_ = consts.tile([128, kc, mdim], BF, tag=name)
                nc.scalar.dma_start(t_[:], ap.rearrange("k p m -> p k m"))
                w_sb[name] = t_

            ones_row = consts.tile([1, 128], BF, tag="ones")
            nc.gpsimd.memset(ones_row[:], 1.0)
            identity = consts.tile([128, 128], F32, tag="ident")
            nc.gpsimd.memset(identity[:], 0.0)
            nc.gpsimd.affine_select(
                out=identity[:],
                in_=identity[:],
                compare_op=OP.not_equal,
                fill=1.0,
                base=0,
                pattern=[[-1, 128]],
                channel_multiplier=1,
            )
            ident_f8 = consts.tile([128, 128], F8, tag="identf8")
            nc.gpsimd.memset(ident_f8[:], 0.0)
            nc.gpsimd.affine_select(
                out=ident_f8[:],
                in_=ident_f8[:],
                compare_op=OP.not_equal,
                fill=1.0,
                base=0,
                pattern=[[-1, 128]],
                channel_multiplier=1,
            )
            # prewarm the sigmoid/tanh ACT table during phase 1
            warm = consts.tile([128, 1], F32, tag="warm")
            nc.scalar.activation(warm[:], identity[:, 0:1], AF.Sigmoid)
            gwarm = consts.tile([128, 128], BF, tag="gwarm")
            nc.gpsimd.tensor_tensor(gwarm[:], ident_f8[:], ident_f8[:],
                                    OP.add)

            # PE keepalive: junk matmuls emitted at known stall points keep
            # the tensor engine's p-state at max (idle gaps downclock it and
            # the next real matmuls run 2-4x slower).
            jk_ps = psum.tile([128, 512], F32, tag="junk", bufs=1)

            def fill(n):
                for _ in range(n):
                    nc.tensor.matmul(jk_ps[:, 0:64], gwarm[:],
                                     gwarm[:, 0:64], start=True, stop=True)

            def fill_t(tsrc, n):
                # junk matmuls reading a chain tile: become ready when the
                # chain op lands, bridging PE idle before the next real
                # matmul group (p-state keepalive)
                for _ in range(n):
                    nc.tensor.matmul(jk_ps[:, 0:256], gwarm[:],
                                     tsrc[:, 0:256], start=True, stop=True)

            def fill_e(e_sb, n):
                # junk matmuls whose moving operand is a just-arrived enc
                # chunk: they become ready exactly when the chunk lands,
                # bridging PE idle between chunk arrivals (p-state keepalive)
                for _ in range(n):
                    nc.tensor.matmul(jk_ps[:], ident_f8[:],
                                     e_sb[:, 0:512], start=True, stop=True)

            # ---- phase 1: zone-mean; DVE pairwise tree for most chunks,
            # PE identity-matmul accumulation for PE_CHUNKS (the PE is
            # otherwise idle during the stream; PSUM accumulates in f32)
            tmpf = state.tile([128, 512], F32, tag="tmpf")
            ptl = state.tile([128, 256], F32, tag="ptl")
            acc = state.tile([128, 256], F32, tag="acc")
            zsum = psum.tile([128, 256], F32, tag="outp", bufs=2)
            n_pe = sum(ZCHS[i] for i in PE_CHUNKS)
            pe_grp = _Group(n_pe)
            first_dve = True
            for i in range(NCH):
                e_sb = e_tiles[i]
                if i in PE_CHUNKS:
                    for z in range(ZCHS[i]):
                        st, sp = pe_grp.flags()
                        nc.tensor.matmul(
                            zsum[:], ident_f8[:],
                            e_sb[:, z * D : (z + 1) * D],
                            start=st, stop=sp,
                        )
                    continue
                w = ZCHS[i] * D
                scr = state.tile([128, 16 * D], BF, tag="scr")
                h = w // 2
                nc.vector.tensor_tensor(
                    scr[:, 0:h], e_sb[:, 0:h], e_sb[:, h:w], OP.add
                )
                w = h
                while w > 4 * D:
                    h = w // 2
                    nc.vector.tensor_tensor(
                        scr[:, 0:h], scr[:, 0:h], scr[:, h:w], OP.add
                    )
                    w = h
                nc.vector.tensor_tensor(
                    tmpf[:], scr[:, 0 : 2 * D], scr[:, 2 * D : 4 * D], OP.add
                )
                if first_dve:
                    nc.vector.tensor_tensor(
                        acc[:], tmpf[:, 0:D], tmpf[:, D : 2 * D], OP.add
                    )
                    first_dve = False
                else:
                    nc.vector.tensor_tensor(
                        ptl[:], tmpf[:, 0:D], tmpf[:, D : 2 * D], OP.add
                    )
                    nc.vector.tensor_tensor(acc[:], acc[:], ptl[:], OP.add)
            ztot = state.tile([128, 256], F32, tag="ztot")
            nc.scalar.activation(ztot[:], zsum[:], AF.Copy)

            # ---- state: actT slots [h0c0, h0c1, h1c0, h1c1] ----
            # the acc + ztot merge rides the PSUM accumulation of the
            # two transposes
            actT = state.tile([128, 4, 128], BF, tag="actT")
            for c in range(2):
                cs = slice(c * 128, (c + 1) * 128)
                ctps = psum.tile([128, 128], F32, tag="outp", bufs=2)
                nc.tensor.matmul(ctps[:], acc[:, cs], identity[:],
                                 is_transpose=True, start=True, stop=False)
                nc.tensor.matmul(ctps[:], ztot[:, cs], identity[:],
                                 is_transpose=True, start=False, stop=True)
                nc.scalar.activation(actT[:, c, :], ctps[:], AF.Copy,
                                     scale=1.0 / NZ)
                nc.scalar.activation(actT[:, 2 + c, :], ctps[:], AF.Copy,
                                     scale=1.0 / NZ)

            # ---- decode-phase emitters ----
            def gate_mms(g, grp, w_t, kis, slots, mlo, mhi, coloff=0):
                """slots entries: int -> actT slot; (tile, k) -> gates tile
                chunk k used as the moving operand."""
                for m in range(mlo, mhi):
                    ms = slice((coloff + m) * 128, (coloff + m + 1) * 128)
                    wms = slice(m * 128, (m + 1) * 128)
                    for ki, slot in zip(kis, slots):
                        if isinstance(slot, tuple):
                            src, k = slot
                            rhs = src[:, k * 128 : (k + 1) * 128]
                        else:
                            rhs = actT[:, slot, :]
                        st, sp = grp.flags()
                        nc.tensor.matmul(
                            g[:, ms], w_t[:, ki, wms], rhs, start=st, stop=sp,
                        )

            # ---- phase 2: 12 decode steps ----
            cur = {}

            def bias_mm(g, grp, gidx):
                """Start a bank's single accumulation group by adding all
                four 128-chunk biases via one K=8 matmul against a one-hot
                moving operand (rows 0:4 bf16-hi, 4:8 bf16-lo)."""
                st, sp = grp.flags()
                nc.tensor.matmul(
                    g[:, 0:512], biasT_sb[0:8, gidx * 128 : (gidx + 1) * 128],
                    oh8_sb[0:8, 0:512], start=st, stop=sp,
                )

            def emit_pre0(t):
                """rz0-hh / hn0 / biases for step t: depend only on
                h0(t-1)."""
                g_rz0 = psum.tile([128, 512], F32, tag="rz0", bufs=2)
                g_hi0 = psum.tile([128, 512], F32, tag="hi0", bufs=1)
                grz0 = _Group(1 + 8 + (12 if t > 0 else 0))
                ghi0 = _Group(1 + 4 + (4 if t > 0 else 0))
                bias_mm(g_rz0, grz0, BT_RZ0 + t)
                bias_mm(g_hi0, ghi0, BT_HI0 + t)
                gate_mms(g_rz0, grz0, w_sb["wrz0"], (2, 3), (0, 1), 0, 4)
                gate_mms(g_hi0, ghi0, w_sb["whn0"], (0, 1), (0, 1), 0, 2)
                cur[t] = (g_rz0, g_hi0, grz0, ghi0)

            fill(FILL_RAMP)
            emit_pre0(0)
            prev_w = None
            prev_cq = None   # (c_, q_) of the most recent layer-1 chain

            def emit_wout_mms():
                """wout matmuls on h1 (read actT slots 2,3 -- must be
                emitted before layer-1 overwrites them)."""
                g_pb = psum.tile([128, 256], F32, tag="outp", bufs=2)
                gout = _Group(3)
                st, sp = gout.flags()
                nc.tensor.matmul(
                    g_pb[:], ones_row[:],
                    brow_sb[0:1, _BOUT * 128 : (_BOUT + 2) * 128],
                    start=st, stop=sp,
                )
                for ki, slot in ((0, 2), (1, 3)):
                    st, sp = gout.flags()
                    nc.tensor.matmul(
                        g_pb[:], actT[:, slot, :], w_sb["wout"][:, ki, :],
                        start=st, stop=sp,
                    )
                return g_pb

            def emit_wout_copy(g_pb, t):
                """PSUM->SBUF stage + output DMA; emitted at the end of the
                layer-1 block so the ACT COPY queues after the critical
                tanh, not before it."""
                o_ = ostage.tile([128, 256], BF, tag="ost")
                nc.scalar.activation(o_[:], g_pb[:], AF.Copy)
                nc.sync.dma_start(out[:, t, :], o_[:])

            for t in range(nsteps):
                g_rz0, g_hi0, grz0, ghi0 = cur.pop(t)
                g_rz1 = psum.tile([128, 512], F32, tag="rz1", bufs=1)
                g_hi1 = psum.tile([128, 512], F32, tag="hi1", bufs=1)
                grz1 = _Group(1 + 8 + 12)
                ghi1 = _Group(1 + 4 + 4)

                for layer in range(2):
                    if layer == 0:
                        g_rz, g_hi, grz, ghi = g_rz0, g_hi0, grz0, ghi0
                        h_sl, x_sl = 0, (2, 3)      # h slots; x = other h
                        w_f, w_in = w_sb["wrz0"], w_sb["win0"]
                        cq = prev_cq if t > 0 else None
                    else:
                        g_rz, g_hi, grz, ghi = g_rz1, g_hi1, grz1, ghi1
                        h_sl, x_sl = 2, (0, 1)
                        w_f, w_in = w_sb["wrz1"], w_sb["win1"]
                        cq = this_cq  # layer-0 chain of this step

                    # r-part of the input block via W@c + (-W)@q: fires as
                    # soon as the previous chain's c/q exist (before h');
                    # in-gate mms right behind so gin is in PSUM before v_
                    s_ = gates.tile([128, 512], BF, tag=f"s{layer}")
                    hnb = gates.tile([128, 256], BF, tag=f"hb{layer}")
                    if cq is not None:
                        c_p, q_p = cq
                        fill(FILL_CQ)
                        gate_mms(g_rz, grz, w_f, (0, 1),
                                 ((c_p, 0), (c_p, 1)), 0, 2)
                        gate_mms(g_rz, grz, w_f, (0, 1),
                                 ((q_p, 0), (q_p, 1)), 0, 2)
                    # stage the hn gate to SBUF bf16 on the DVE; emitted
                    # BEFORE the in-mms so its (tile-granular) g_hi dep
                    # closes at the long-done hn mms -> runs in DVE idle
                    nc.vector.tensor_scalar_add(hnb[:], g_hi[:, 0:256], 0.0)
                    if cq is not None:
                        gate_mms(g_hi, ghi, w_in, (0, 1), x_sl, 0, 2,
                                 coloff=2)
                    # biases pre-accumulated into PSUM: single 256-wide ACTs
                    nc.scalar.activation(s_[:, 0:256], g_rz[:, 0:256],
                                         AF.Sigmoid)
                    # z-part plainly on h' of the previous chain
                    if cq is not None:
                        gate_mms(g_rz, grz, w_f, (0, 1), x_sl, 2, 4)
                    nc.scalar.activation(s_[:, 256:512], g_rz[:, 256:512],
                                         AF.Sigmoid)
                    t_ = gates.tile([128, 256], BF, tag=f"t{layer}")
                    nc.vector.tensor_tensor(t_[:], hnb[:], s_[:, 0:256],
                                            OP.mult)
                    if layer == 0:
                        # layer-1 parts that need only h1(t-1)
                        bias_mm(g_rz1, grz1, BT_RZ1)
                        bias_mm(g_hi1, ghi1, BT_HI1)
                        gate_mms(g_rz1, grz1, w_sb["wrz1"], (2, 3), (2, 3),
                                 0, 4)
                        gate_mms(g_hi1, ghi1, w_sb["whn1"], (0, 1), (2, 3),
                                 0, 2)
                    else:
                        # step t+1 parts that need only h0'(t)
                        if t + 1 < nsteps:
                            emit_pre0(t + 1)
                    v_ = gates.tile([128, 256], BF, tag=f"v{layer}")
                    nc.vector.scalar_tensor_tensor(
                        v_[:], g_hi[:, 256:512], 0.0, t_[:],
                        op0=OP.add, op1=OP.add,
                    )
                    # 256-wide tail: n/c/zm1/q/h'.  q = (z-1)*n as a 2x TT
                    # against the precomputed zm1 (the STT form runs 1x).
                    c_ = gates.tile([128, 256], BF, tag=f"c{layer}")
                    n_ = gates.tile([128, 256], BF, tag=f"n{layer}")
                    q_ = gates.tile([128, 256], BF, tag=f"q{layer}")
                    zm1 = gates.tile([128, 256], BF, tag=f"z{layer}")
                    nc.scalar.activation(n_[:], v_[:], AF.Tanh)
                    nc.vector.tensor_tensor(
                        c_[:], s_[:, 256:512], actT[:, h_sl : h_sl + 2, :],
                        OP.mult,
                    )
                    nc.vector.tensor_scalar(zm1[:], s_[:, 256:512], 1.0,
                                            -1.0, op0=OP.subtract,
                                            op1=OP.mult)
                    nc.vector.tensor_tensor(q_[:], zm1[:], n_[:], OP.mult)
                    nc.vector.tensor_tensor(
                        actT[:, h_sl : h_sl + 2, :], c_[:], q_[:],
                        OP.add,
                    )

                    fill_t(v_, 1)
                    fill_t(n_, 1)
                    fill_t(zm1, 1)
                    if layer == 0:
                        this_cq = (c_, q_)
                        if prev_w is not None:
                            prev_pb = emit_wout_mms()
                    else:
                        prev_cq = (c_, q_)
                        if prev_w is not None:
                            emit_wout_copy(prev_pb, prev_w)
                prev_w = t
            g_pb = emit_wout_mms()
            emit_wout_copy(g_pb, prev_w)

    _split_waits(nc)
    return nc


def _prep_inputs(encoded_features, step_emb, W_ih0, W_hh0, b_ih0, b_hh0,
                 W_ih1, W_hh1, b_ih1, b_hh1, W_out, b_out):
    """Host-side: slice/shard the big input, transpose + cast weights,
    fold the output projection into layer-0 input weights, fold the
    step-embedding matmul + all additive constants into bias columns."""
    f4 = np.float32
    enc_last = np.asarray(encoded_features)[:, -1].astype(ml_dtypes.float8_e4m3)
    enc_last = np.ascontiguousarray(enc_last)

    W_ih0 = np.asarray(W_ih0, f4)
    W_hh0 = np.asarray(W_hh0, f4)
    W_ih1 = np.asarray(W_ih1, f4)
    W_hh1 = np.asarray(W_hh1, f4)
    W_out = np.asarray(W_out, f4)
    step_emb = np.asarray(step_emb, f4)
    b_ih0 = np.asarray(b_ih0, f4)
    b_hh0 = np.asarray(b_hh0, f4)
    b_ih1 = np.asarray(b_ih1, f4)
    b_hh1 = np.asarray(b_hh1, f4)
    b_out = np.asarray(b_out, f4)

    W_emb = W_ih0[:, :D]          # (768, 256)
    W_pred = W_ih0[:, D:]         # (768, 256)
    W_fold = W_pred @ W_out       # (768, 256): pred feedback folded onto h1
    b_fold = W_pred @ b_out       # (768,)

    # gi_emb[t] = W_emb @ step_emb[t] + b_ih0  -> (12, 768)
    gi_emb = step_emb[:STEPS] @ W_emb.T + b_ih0[None, :]

    def kstack(*mats_cols):
        chunks = []
        for mat, cols in mats_cols:
            mt = np.ascontiguousarray(mat.T[:, cols])  # (K, M)
            for k in range(0, mt.shape[0], 128):
                chunks.append(mt[k : k + 128])
        return np.stack(chunks).astype(BF16)  # (nk, 128, M)

    rz = slice(0, 512)
    ng = slice(512, 768)
    wrz0 = kstack((W_fold, rz), (W_hh0, rz))          # K: h1c0,h1c1,h0c0,h0c1
    win0 = kstack((W_fold, ng))
    whn0 = kstack((W_hh0, ng))
    wrz1 = kstack((W_ih1, rz), (W_hh1, rz))           # K: h0c0,h0c1,h1c0,h1c1
    win1 = kstack((W_ih1, ng))
    whn1 = kstack((W_hh1, ng))
    wout = np.stack([np.ascontiguousarray(W_out.T)[k : k + 128] for k in (0, 128)]
                    ).astype(BF16)                    # (2, 128, 256)

    biasT = np.zeros((8, NGRP * 128), f4)

    def putg(g, vec):
        # bf16 hi/lo split: rows 0:nk hold bf16(vec) chunks, nk:2nk residual
        nk = len(vec) // 128
        hi = vec.astype(BF16).astype(f4)
        lo = vec - hi
        for k in range(nk):
            biasT[k, g * 128 : (g + 1) * 128] = hi[k * 128 : (k + 1) * 128]
            biasT[nk + k, g * 128 : (g + 1) * 128] = (
                lo[k * 128 : (k + 1) * 128])

    for t in range(STEPS):
        extra = b_fold if t > 0 else np.zeros_like(b_fold)
        putg(BT_RZ0 + t, gi_emb[t, :512] + b_hh0[:512] + extra[:512])
        putg(BT_HI0 + t, np.concatenate(
            [b_hh0[512:], gi_emb[t, 512:] + extra[512:]]))
    putg(BT_RZ1, b_ih1[:512] + b_hh1[:512])
    putg(BT_HI1, np.concatenate([b_hh1[512:], b_ih1[512:]]))
    biasT = biasT.astype(BF16)

    oh8 = np.zeros((8, 512), f4)
    for k in range(8):
        oh8[k, (k % 4) * 128 : (k % 4 + 1) * 128] = 1.0
    oh8 = oh8.astype(BF16)

    brows = np.zeros(NROW * 128, f4)
    brows[_BOUT * 128 : _BOUT * 128 + 256] = b_out
    brows = brows.astype(BF16)[None, :]

    shared = dict(wrz0=wrz0, win0=win0, whn0=whn0, wrz1=wrz1,
                  win1=win1, whn1=whn1, wout=wout,
                  biasT=biasT, oh8=oh8, brows=brows)
    in_maps = []
    for i in range(N_CORES):
        m = dict(shared)
        m["enc"] = enc_last[i * PC : (i + 1) * PC]
        in_maps.append(m)
    return in_maps


_CACHE = {}


def _run(in_maps, trace=False):
    from concourse.bass_utils import run_bass_kernel_spmd

    if "nc" not in _CACHE:
        _CACHE["nc"] = build_kernel()
    nc = _CACHE["nc"]
    res = run_bass_kernel_spmd(
        nc, in_maps, core_ids=list(range(N_CORES)), trace=trace
    )
    preds = np.concatenate([res.results[i]["out"] for i in range(N_CORES)],
                       axis=0).astype(np.float32)
    return preds, res


def kernel(encoded_features, step_emb, W_ih0, W_hh0, b_ih0, b_hh0,
           W_ih1, W_hh1, b_ih1, b_hh1, W_out, b_out, num_steps):
    assert int(num_steps) == STEPS
    in_maps = _prep_inputs(encoded_features, step_emb, W_ih0, W_hh0, b_ih0,
                           b_hh0, W_ih1, W_hh1, b_ih1, b_hh1, W_out, b_out)
    preds, _ = _run(in_maps, trace=False)
    return preds



# revision 47
# speedup vs baseline: 1.0336x; 1.0336x over previous
"""Trainium2 Bass kernel for the autoregressive GRU decode head.

Problem: context = mean over zones of encoded_features[:, -1]  -> (B, D)
then 12 autoregressive steps of a 2-layer GRU (H=256) + linear projection
to N=256 zones.  B=1024, data-parallel across 8 NeuronCores (128 batch each).

Structure (per core, feature-major / "transposed" activations):
  actT (128p, 4 slots, 128) bf16 : [h0 c0, h0 c1, h1 c0, h1 c1]
       slot holds h[b, c*128 + p] at [p, b]    (c = chunk of the 256-dim)
  Gate tensors (PSUM) use layout [p, c*128 + b].
  Matmuls: out(gate_chunk, B) = lhsT.T @ rhs, lhsT = W^T tile (K<=128, M=128),
  rhs = actT slot (K=128, B=128), K-chunks accumulated in PSUM.
  The prediction feedback is algebraically folded into layer 0's weights:
  W_pred @ (W_out h1 + b_out) = (W_pred W_out) h1 + W_pred b_out.
  Chain per layer (PSUM g_rz = [r|z], g_hi = [ghn|gin]):
    r,z = sigmoid(g_rz + bias)   4x 128-wide ACT (bias via ACT operand)
    t = (ghn + bhn) * r          STT 256w
    v = (gin + bin) + t          STT 256w
    n = tanh(v); c = z*h; q = (z-1)*n; h' = c - q
  Critical-path trick: the NEXT matmul block needs W @ h' with h' = c - q,
  so it is issued as W @ c + (-W) @ q against the c/q tiles, which exist
  ~1us before h' does -- the r-gate matmuls of the next layer/step complete
  almost immediately after the chain, instead of serializing behind h'.
  The z / in-gate matmuls use h' directly (they are needed later).
  All h1(t-1)-only work runs during chain0; all h0'(t)-only work (incl.
  step t+1's rz0/hn0 h-parts) during chain1; wout(t-1) during chain0.
The encoded_features slice is streamed as bf16 (host-converted) in 6 chunks
(64,64,64,32,16,16 zones) ALL on the sync HWDGE queue so arrival order
matches the tree's processing order; weights/biases ride the scalar queue.
The zone mean is a DVE-only pairwise TT-add tree (bf16 upper levels, f32
tail) overlapped with the DMA.  GpSimd is avoided: it shares an SBUF port
with the DVE and large concurrent ops slow both ~4x.
"""

import sys

for _p in ("/opt/trn_rl_repo",):
    if _p not in sys.path:
        sys.path.insert(0, _p)

import numpy as np
import ml_dtypes

import concourse.bass as bass
import concourse.tile as tile
from concourse import mybir
from concourse.vector_clock import ScopedClock

BF16 = ml_dtypes.bfloat16

B, T, NZ, D = 1024, 8, 256, 256
H = 256
STEPS = 12
N_CORES = 8
PC = B // N_CORES  # 128 batch per core

F32 = mybir.dt.float32
BF = mybir.dt.bfloat16
F8 = mybir.dt.float8e4
AF = mybir.ActivationFunctionType
OP = mybir.AluOpType

# biasT (bf16, [8, NGRP*128]): per-group bias stacks for the K=8 bias
# matmuls (one per PSUM bank+step: a start=True matmul resets the WHOLE
# bank, so each bank gets exactly one group whose first matmul adds all
# four 128-chunk biases).  Group g's lhsT is biasT[0:8, g*128:(g+1)*128];
# rows 0:4 hold bf16-hi chunk biases, rows 4:8 the bf16-lo residuals.
BT_RZ0 = 0                    # 12 steps: [r c0, r c1, z c0, z c1] of L0
BT_RZ1 = BT_RZ0 + STEPS
BT_HI0 = BT_RZ1 + 1           # 12 steps: [hn c0, hn c1, in c0, in c1]
BT_HI1 = BT_HI0 + STEPS
NGRP = BT_HI1 + 1

# brow (bf16, [1, 2*128]): b_out as a moving row for the batch-major wout
_BOUT = 0
NROW = 2


def _install_tile_drain_patch():
    """walrus (CoreV3) rejects >1 sync wait on the tail drain; spill extras
    onto preceding sync nops."""
    if getattr(tile.TileContext, "_drain_patch_installed", False):
        return

    def _patched(self, tick_clock, wait_clock):
        nc = self.nc
        bb = nc.cur_bb.bb
        drain_bi = nc.sync.drain()
        drain_inst = drain_bi.ins
        wait_clock.add_sem_waits(
            drain_inst, ScopedClock({None: tick_clock.global_clock})
        )
        w = drain_inst.sync_info.on_wait if drain_inst.sync_info else None
        maxw = 1
        if w and len(w) > maxw:
            extra = list(w[maxw:])
            drain_inst.sync_info.on_wait = list(w[:maxw])
            idx = bb.instructions.index(drain_inst)
            nops = []
            for i in range(0, len(extra), maxw):
                nop_bi = nc.sync.nop()
                nop = nop_bi.ins
                si = nop.sync_info
                nop.sync_info = mybir.SyncInfo(
                    on_wait=extra[i : i + maxw],
                    on_update=(si.on_update if si else []),
                )
                bb.instructions.remove(nop)
                nops.append(nop)
            bb.instructions[idx:idx] = nops
        nc.all_engine_barrier()
        popped = nc._tile_sem_poison_stack.pop()
        assert popped is self._sem_poison
        nc.clear_and_free_semaphores(list(self.sems.allocated().values()))
        nc.all_engine_barrier()

    tile.TileContext._drain_and_barrier = _patched
    tile.TileContext._drain_patch_installed = True


def _split_waits(nc, maxw=1):
    """This walrus build rejects instructions carrying more than ~1 sem
    wait; spill extra waits onto same-engine nops placed just before."""
    for bb in nc.main_func.blocks:
        new_list = []
        for inst in bb.instructions:
            si = inst.sync_info
            w = list(si.on_wait) if si and si.on_wait else []
            if len(w) > maxw:
                keep = w[len(w) - maxw:]
                extra = w[: len(w) - maxw]
                si.on_wait = keep
                for i in range(0, len(extra), maxw):
                    nop = mybir.InstNoOp(
                        name=f"{inst.name}-sw{i}", ins=[], outs=[]
                    )
                    nop.engine = inst.engine
                    nop.sync_info = mybir.SyncInfo(
                        on_wait=extra[i : i + maxw], on_update=[]
                    )
                    nc.register_instruction(nop)
                    new_list.append(nop)
            new_list.append(inst)
        bb.instructions[:] = new_list


class _Group:
    """Tracks start/stop flags for a PSUM accumulation group whose matmuls
    are emitted in several program-order batches."""

    def __init__(self, total):
        self.total = total
        self.emitted = 0

    def flags(self):
        start = self.emitted == 0
        self.emitted += 1
        return start, self.emitted == self.total


FILL_CQ = 0     # junk matmuls before each layer.s c/q-dependent block
FILL_P1 = 0      # before each phase-1 PE zone-sum chunk group
FILL_RAMP = 0   # bridging the phase-1 -> decode transition


def build_kernel(nsteps=12):
    """Build the per-core Bass graph (SPMD: same graph on all 8 cores)."""
    _install_tile_drain_patch()
    nc = bass.Bass()

    enc = nc.declare_dram_parameter("enc", [PC, NZ, D], F8, isOutput=False)
    wrz0 = nc.declare_dram_parameter("wrz0", [4, 128, 512], BF, isOutput=False)
    win0 = nc.declare_dram_parameter("win0", [2, 128, 256], BF, isOutput=False)
    whn0 = nc.declare_dram_parameter("whn0", [2, 128, 256], BF, isOutput=False)
    wrz1 = nc.declare_dram_parameter("wrz1", [4, 128, 512], BF, isOutput=False)
    win1 = nc.declare_dram_parameter("win1", [2, 128, 256], BF, isOutput=False)
    whn1 = nc.declare_dram_parameter("whn1", [2, 128, 256], BF, isOutput=False)
    wout = nc.declare_dram_parameter("wout", [2, 128, 256], BF, isOutput=False)
    biasT = nc.declare_dram_parameter("biasT", [8, NGRP * 128], BF,
                                      isOutput=False)
    oh8 = nc.declare_dram_parameter("oh8", [8, 512], BF, isOutput=False)
    brows = nc.declare_dram_parameter("brows", [1, NROW * 128], BF,
                                      isOutput=False)
    out = nc.declare_dram_parameter("out", [PC, STEPS, NZ], BF, isOutput=True)

    with tile.TileContext(nc) as tc:
        with (
            tc.tile_pool(name="consts", bufs=1) as consts,
            tc.tile_pool(name="state", bufs=1) as state,
            tc.tile_pool(name="enc_pool", bufs=4) as enc_pool,
            tc.tile_pool(name="gates", bufs=2) as gates,
            tc.tile_pool(name="ostage", bufs=2) as ostage,
            tc.tile_pool(name="psum", bufs=1, space="PSUM") as psum,
        ):
            # ---- phase 1 DMA: enc all on the sync queue, in tree order ----
            # PE consumes ~2x faster than the DVE tree: it owns the first
            # chunks plus the tail; the DVE takes two early-arriving chunks
            ZCHS = [32, 32, 32, 32, 32, 32, 32, 16, 16]
            PE_CHUNKS = (0, 2, 4, 5, 6, 7, 8)
            NCH = len(ZCHS)
            # enc split across both HWDGE queues (even->sync, odd->scalar)
            # so the two queues' per-chunk completion overheads overlap and
            # the aggregate stream runs at the DMA bandwidth roofline
            e_tiles = []
            z0 = 0
            for i, zch in enumerate(ZCHS):
                e_sb = enc_pool.tile([128, 32 * D], F8, tag="echunk",
                                     bufs=13)
                eng = nc.sync if i % 2 == 0 else nc.scalar
                eng.dma_start(e_sb[:, : zch * D], enc[:, z0 : z0 + zch, :])
                e_tiles.append(e_sb)
                z0 += zch

            # small constants then weights behind the enc stream, ordered
            # by first use
            biasT_sb = consts.tile([8, NGRP * 128], BF, tag="biasT")
            nc.scalar.dma_start(biasT_sb[:], biasT[:])
            oh8_sb = consts.tile([8, 512], BF, tag="oh8")
            nc.scalar.dma_start(oh8_sb[:], oh8[:])
            brow_sb = consts.tile([1, NROW * 128], BF, tag="brow")
            nc.scalar.dma_start(brow_sb[:], brows[:])
            w_sb = {}
            for name, ap, kc, mdim in (
                ("wrz0", wrz0, 4, 512),
                ("whn0", whn0, 2, 256),
                ("wrz1", wrz1, 4, 512),
                ("whn1", whn1, 2, 256),
                ("win1", win1, 2, 256),
                ("wout", wout, 2, 256),
                ("win0", win0, 2, 256),
            ):
                t_ = consts.tile([128, kc, mdim], BF, tag=name)
                nc.scalar.dma_start(t_[:], ap.rearrange("k p m -> p k m"))
                w_sb[name] = t_

            ones_row = consts.tile([1, 128], BF, tag="ones")
            nc.gpsimd.memset(ones_row[:], 1.0)
            identity = consts.tile([128, 128], F32, tag="ident")
            nc.gpsimd.memset(identity[:], 0.0)
            nc.gpsimd.affine_select(
                out=identity[:],
                in_=identity[:],
                compare_op=OP.not_equal,
                fill=1.0,
                base=0,
                pattern=[[-1, 128]],
                channel_multiplier=1,
            )
            ident_f8 = consts.tile([128, 128], F8, tag="identf8")
            nc.gpsimd.memset(ident_f8[:], 0.0)
            nc.gpsimd.affine_select(
                out=ident_f8[:],
                in_=ident_f8[:],
                compare_op=OP.not_equal,
                fill=1.0,
                base=0,
                pattern=[[-1, 128]],
                channel_multiplier=1,
            )
            # prewarm the sigmoid/tanh ACT table during phase 1
            warm = consts.tile([128, 1], F32, tag="warm")
            nc.scalar.activation(warm[:], identity[:, 0:1], AF.Sigmoid)
            gwarm = consts.tile([128, 128], BF, tag="gwarm")
            nc.gpsimd.tensor_tensor(gwarm[:], ident_f8[:], ident_f8[:],
                                    OP.add)

            # PE keepalive: junk matmuls emitted at known stall points keep
            # the tensor engine's p-state at max (idle gaps downclock it and
            # the next real matmuls run 2-4x slower).
            jk_ps = psum.tile([128, 512], F32, tag="junk", bufs=1)

            def fill(n):
                for _ in range(n):
                    nc.tensor.matmul(jk_ps[:, 0:64], gwarm[:],
                                     gwarm[:, 0:64], start=True, stop=True)

            def fill_t(tsrc, n):
                # junk matmuls reading a chain tile: become ready when the
                # chain op lands, bridging PE idle before the next real
                # matmul group (p-state keepalive)
                for _ in range(n):
                    nc.tensor.matmul(jk_ps[:, 0:256], gwarm[:],
                                     tsrc[:, 0:256], start=True, stop=True)

            def fill_e(e_sb, n):
                # junk matmuls whose moving operand is a just-arrived enc
                # chunk: they become ready exactly when the chunk lands,
                # bridging PE idle between chunk arrivals (p-state keepalive)
                for _ in range(n):
                    nc.tensor.matmul(jk_ps[:], ident_f8[:],
                                     e_sb[:, 0:512], start=True, stop=True)

            # ---- phase 1: zone-mean; DVE pairwise tree for most chunks,
            # PE identity-matmul accumulation for PE_CHUNKS (the PE is
            # otherwise idle during the stream; PSUM accumulates in f32)
            tmpf = state.tile([128, 512], F32, tag="tmpf")
            ptl = state.tile([128, 256], F32, tag="ptl")
            acc = state.tile([128, 256], F32, tag="acc")
            zsum = psum.tile([128, 256], F32, tag="outp", bufs=2)
            n_pe = sum(ZCHS[i] for i in PE_CHUNKS)
            pe_grp = _Group(n_pe)
            first_dve = True
            for i in range(NCH):
                e_sb = e_tiles[i]
                if i in PE_CHUNKS:
                    for z in range(ZCHS[i]):
                        st, sp = pe_grp.flags()
                        nc.tensor.matmul(
                            zsum[:], ident_f8[:],
                            e_sb[:, z * D : (z + 1) * D],
                            start=st, stop=sp,
                        )
                    continue
                w = ZCHS[i] * D
                scr = state.tile([128, 16 * D], BF, tag="scr")
                h = w // 2
                nc.vector.tensor_tensor(
                    scr[:, 0:h], e_sb[:, 0:h], e_sb[:, h:w], OP.add
                )
                w = h
                while w > 4 * D:
                    h = w // 2
                    nc.vector.tensor_tensor(
                        scr[:, 0:h], scr[:, 0:h], scr[:, h:w], OP.add
                    )
                    w = h
                nc.vector.tensor_tensor(
                    tmpf[:], scr[:, 0 : 2 * D], scr[:, 2 * D : 4 * D], OP.add
                )
                if first_dve:
                    nc.vector.tensor_tensor(
                        acc[:], tmpf[:, 0:D], tmpf[:, D : 2 * D], OP.add
                    )
                    first_dve = False
                else:
                    nc.vector.tensor_tensor(
                        ptl[:], tmpf[:, 0:D], tmpf[:, D : 2 * D], OP.add
                    )
                    nc.vector.tensor_tensor(acc[:], acc[:], ptl[:], OP.add)
            ztot = state.tile([128, 256], F32, tag="ztot")
            nc.scalar.activation(ztot[:], zsum[:], AF.Copy)

            # ---- state: actT slots [h0c0, h0c1, h1c0, h1c1] ----
            # the acc + ztot merge rides the PSUM accumulation of the
            # two transposes
            actT = state.tile([128, 4, 128], BF, tag="actT")
            for c in range(2):
                cs = slice(c * 128, (c + 1) * 128)
                ctps = psum.tile([128, 128], F32, tag="outp", bufs=2)
                nc.tensor.matmul(ctps[:], acc[:, cs], identity[:],
                                 is_transpose=True, start=True, stop=False)
                nc.tensor.matmul(ctps[:], ztot[:, cs], identity[:],
                                 is_transpose=True, start=False, stop=True)
                nc.scalar.activation(actT[:, c, :], ctps[:], AF.Copy,
                                     scale=1.0 / NZ)
                nc.scalar.activation(actT[:, 2 + c, :], ctps[:], AF.Copy,
                                     scale=1.0 / NZ)

            # ---- decode-phase emitters ----
            def gate_mms(g, grp, w_t, kis, slots, mlo, mhi, coloff=0):
                """slots entries: int -> actT slot; (tile, k) -> gates tile
                chunk k used as the moving operand."""
                for m in range(mlo, mhi):
                    ms = slice((coloff + m) * 128, (coloff + m + 1) * 128)
                    wms = slice(m * 128, (m + 1) * 128)
                    for ki, slot in zip(kis, slots):
                        if isinstance(slot, tuple):
                            src, k = slot
                            rhs = src[:, k * 128 : (k + 1) * 128]
                        else:
                            rhs = actT[:, slot, :]
                        st, sp = grp.flags()
                        nc.tensor.matmul(
                            g[:, ms], w_t[:, ki, wms], rhs, start=st, stop=sp,
                        )

            # ---- phase 2: 12 decode steps ----
            cur = {}

            def bias_mm(g, grp, gidx):
                """Start a bank's single accumulation group by adding all
                four 128-chunk biases via one K=8 matmul against a one-hot
                moving operand (rows 0:4 bf16-hi, 4:8 bf16-lo)."""
                st, sp = grp.flags()
                nc.tensor.matmul(
                    g[:, 0:512], biasT_sb[0:8, gidx * 128 : (gidx + 1) * 128],
                    oh8_sb[0:8, 0:512], start=st, stop=sp,
                )

            def emit_pre0(t):
                """rz0-hh / hn0 / biases for step t: depend only on
                h0(t-1)."""
                g_rz0 = psum.tile([128, 512], F32, tag="rz0", bufs=2)
                g_hi0 = psum.tile([128, 512], F32, tag="hi0", bufs=1)
                grz0 = _Group(1 + 8 + (12 if t > 0 else 0))
                ghi0 = _Group(1 + 4 + (4 if t > 0 else 0))
                bias_mm(g_rz0, grz0, BT_RZ0 + t)
                bias_mm(g_hi0, ghi0, BT_HI0 + t)
                gate_mms(g_rz0, grz0, w_sb["wrz0"], (2, 3), (0, 1), 0, 4)
                gate_mms(g_hi0, ghi0, w_sb["whn0"], (0, 1), (0, 1), 0, 2)
                cur[t] = (g_rz0, g_hi0, grz0, ghi0)

            fill(FILL_RAMP)
            emit_pre0(0)
            prev_w = None
            prev_cq = None   # (c_, q_) of the most recent layer-1 chain

            def emit_wout_mms():
                """wout matmuls on h1 (read actT slots 2,3 -- must be
                emitted before layer-1 overwrites them)."""
                g_pb = psum.tile([128, 256], F32, tag="outp", bufs=2)
                gout = _Group(3)
                st, sp = gout.flags()
                nc.tensor.matmul(
                    g_pb[:], ones_row[:],
                    brow_sb[0:1, _BOUT * 128 : (_BOUT + 2) * 128],
                    start=st, stop=sp,
                )
                for ki, slot in ((0, 2), (1, 3)):
                    st, sp = gout.flags()
                    nc.tensor.matmul(
                        g_pb[:], actT[:, slot, :], w_sb["wout"][:, ki, :],
                        start=st, stop=sp,
                    )
                return g_pb

            def emit_wout_copy(g_pb, t):
                """PSUM->SBUF stage + output DMA; emitted at the end of the
                layer-1 block so the ACT COPY queues after the critical
                tanh, not before it."""
                o_ = ostage.tile([128, 256], BF, tag="ost")
                nc.scalar.activation(o_[:], g_pb[:], AF.Copy)
                nc.sync.dma_start(out[:, t, :], o_[:])

            for t in range(nsteps):
                g_rz0, g_hi0, grz0, ghi0 = cur.pop(t)
                g_rz1 = psum.tile([128, 512], F32, tag="rz1", bufs=1)
                g_hi1 = psum.tile([128, 512], F32, tag="hi1", bufs=1)
                grz1 = _Group(1 + 8 + 12)
                ghi1 = _Group(1 + 4 + 4)

                for layer in range(2):
                    if layer == 0:
                        g_rz, g_hi, grz, ghi = g_rz0, g_hi0, grz0, ghi0
                        h_sl, x_sl = 0, (2, 3)      # h slots; x = other h
                        w_f, w_in = w_sb["wrz0"], w_sb["win0"]
                        cq = prev_cq if t > 0 else None
                    else:
                        g_rz, g_hi, grz, ghi = g_rz1, g_hi1, grz1, ghi1
                        h_sl, x_sl = 2, (0, 1)
                        w_f, w_in = w_sb["wrz1"], w_sb["win1"]
                        cq = this_cq  # layer-0 chain of this step

                    # r-part of the input block via W@c + (-W)@q: fires as
                    # soon as the previous chain's c/q exist (before h');
                    # in-gate mms right behind so gin is in PSUM before v_
                    s_ = gates.tile([128, 512], BF, tag=f"s{layer}")
                    hnb = gates.tile([128, 256], BF, tag=f"hb{layer}")
                    if cq is not None:
                        c_p, q_p = cq
                        fill(FILL_CQ)
                        gate_mms(g_rz, grz, w_f, (0, 1),
                                 ((c_p, 0), (c_p, 1)), 0, 2)
                        gate_mms(g_rz, grz, w_f, (0, 1),
                                 ((q_p, 0), (q_p, 1)), 0, 2)
                    # stage the hn gate to SBUF bf16 on the DVE; emitted
                    # BEFORE the in-mms so its (tile-granular) g_hi dep
                    # closes at the long-done hn mms -> runs in DVE idle
                    nc.vector.tensor_scalar_add(hnb[:], g_hi[:, 0:256], 0.0)
                    if cq is not None:
                        gate_mms(g_hi, ghi, w_in, (0, 1), x_sl, 0, 2,
                                 coloff=2)
                    # biases pre-accumulated into PSUM: single 256-wide ACTs
                    nc.scalar.activation(s_[:, 0:256], g_rz[:, 0:256],
                                         AF.Sigmoid)
                    # z-part plainly on h' of the previous chain
                    if cq is not None:
                        gate_mms(g_rz, grz, w_f, (0, 1), x_sl, 2, 4)
                    nc.scalar.activation(s_[:, 256:512], g_rz[:, 256:512],
                                         AF.Sigmoid)
                    t_ = gates.tile([128, 256], BF, tag=f"t{layer}")
                    nc.vector.tensor_tensor(t_[:], hnb[:], s_[:, 0:256],
                                            OP.mult)
                    if layer == 0:
                        # layer-1 parts that need only h1(t-1)
                        bias_mm(g_rz1, grz1, BT_RZ1)
                        bias_mm(g_hi1, ghi1, BT_HI1)
                        gate_mms(g_rz1, grz1, w_sb["wrz1"], (2, 3), (2, 3),
                                 0, 4)
                        gate_mms(g_hi1, ghi1, w_sb["whn1"], (0, 1), (2, 3),
                                 0, 2)
                    else:
                        # step t+1 parts that need only h0'(t)
                        if t + 1 < nsteps:
                            emit_pre0(t + 1)
                    v_ = gates.tile([128, 256], BF, tag=f"v{layer}")
                    nc.vector.scalar_tensor_tensor(
                        v_[:], g_hi[:, 256:512], 0.0, t_[:],
                        op0=OP.add, op1=OP.add,
                    )
                    # 256-wide tail: n/c/zm1/q/h'.  q = (z-1)*n as a 2x TT
                    # against the precomputed zm1 (the STT form runs 1x).
                    c_ = gates.tile([128, 256], BF, tag=f"c{layer}")
                    n_ = gates.tile([128, 256], BF, tag=f"n{layer}")
                    q_ = gates.tile([128, 256], BF, tag=f"q{layer}")
                    zm1 = gates.tile([128, 256], BF, tag=f"z{layer}")
                    nc.scalar.activation(n_[:], v_[:], AF.Tanh)
                    nc.vector.tensor_tensor(
                        c_[:], s_[:, 256:512], actT[:, h_sl : h_sl + 2, :],
                        OP.mult,
                    )
                    nc.vector.tensor_scalar(zm1[:], s_[:, 256:512], 1.0,
                                            -1.0, op0=OP.subtract,
                                            op1=OP.mult)
                    nc.vector.tensor_tensor(q_[:], zm1[:], n_[:], OP.mult)
                    nc.vector.tensor_tensor(
                        actT[:, h_sl : h_sl + 2, :], c_[:], q_[:],
                        OP.add,
                    )

                    if layer == 0:
                        this_cq = (c_, q_)
                        if prev_w is not None:
                            prev_pb = emit_wout_mms()
                    else:
                        prev_cq = (c_, q_)
                        if prev_w is not None:
                            emit_wout_copy(prev_pb, prev_w)
                prev_w = t
            g_pb = emit_wout_mms()
            emit_wout_copy(g_pb, prev_w)

    _split_waits(nc)
    return nc


def _prep_inputs(encoded_features, step_emb, W_ih0, W_hh0, b_ih0, b_hh0,
                 W_ih1, W_hh1, b_ih1, b_hh1, W_out, b_out):
    """Host-side: slice/shard the big input, transpose + cast weights,
    fold the output projection into layer-0 input weights, fold the
    step-embedding matmul + all additive constants into bias columns."""
    f4 = np.float32
    enc_last = np.asarray(encoded_features)[:, -1].astype(ml_dtypes.float8_e4m3)
    enc_last = np.ascontiguousarray(enc_last)

    W_ih0 = np.asarray(W_ih0, f4)
    W_hh0 = np.asarray(W_hh0, f4)
    W_ih1 = np.asarray(W_ih1, f4)
    W_hh1 = np.asarray(W_hh1, f4)
    W_out = np.asarray(W_out, f4)
    step_emb = np.asarray(step_emb, f4)
    b_ih0 = np.asarray(b_ih0, f4)
    b_hh0 = np.asarray(b_hh0, f4)
    b_ih1 = np.asarray(b_ih1, f4)
    b_hh1 = np.asarray(b_hh1, f4)
    b_out = np.asarray(b_out, f4)

    W_emb = W_ih0[:, :D]          # (768, 256)
    W_pred = W_ih0[:, D:]         # (768, 256)
    W_fold = W_pred @ W_out       # (768, 256): pred feedback folded onto h1
    b_fold = W_pred @ b_out       # (768,)

    # gi_emb[t] = W_emb @ step_emb[t] + b_ih0  -> (12, 768)
    gi_emb = step_emb[:STEPS] @ W_emb.T + b_ih0[None, :]

    def kstack(*mats_cols):
        chunks = []
        for mat, cols in mats_cols:
            mt = np.ascontiguousarray(mat.T[:, cols])  # (K, M)
            for k in range(0, mt.shape[0], 128):
                chunks.append(mt[k : k + 128])
        return np.stack(chunks).astype(BF16)  # (nk, 128, M)

    rz = slice(0, 512)
    ng = slice(512, 768)
    wrz0 = kstack((W_fold, rz), (W_hh0, rz))          # K: h1c0,h1c1,h0c0,h0c1
    win0 = kstack((W_fold, ng))
    whn0 = kstack((W_hh0, ng))
    wrz1 = kstack((W_ih1, rz), (W_hh1, rz))           # K: h0c0,h0c1,h1c0,h1c1
    win1 = kstack((W_ih1, ng))
    whn1 = kstack((W_hh1, ng))
    wout = np.stack([np.ascontiguousarray(W_out.T)[k : k + 128] for k in (0, 128)]
                    ).astype(BF16)                    # (2, 128, 256)

    biasT = np.zeros((8, NGRP * 128), f4)

    def putg(g, vec):
        # bf16 hi/lo split: rows 0:nk hold bf16(vec) chunks, nk:2nk residual
        nk = len(vec) // 128
        hi = vec.astype(BF16).astype(f4)
        lo = vec - hi
        for k in range(nk):
            biasT[k, g * 128 : (g + 1) * 128] = hi[k * 128 : (k + 1) * 128]
            biasT[nk + k, g * 128 : (g + 1) * 128] = (
                lo[k * 128 : (k + 1) * 128])

    for t in range(STEPS):
        extra = b_fold if t > 0 else np.zeros_like(b_fold)
        putg(BT_RZ0 + t, gi_emb[t, :512] + b_hh0[:512] + extra[:512])
        putg(BT_HI0 + t, np.concatenate(
            [b_hh0[512:], gi_emb[t, 512:] + extra[512:]]))
    putg(BT_RZ1, b_ih1[:512] + b_hh1[:512])
    putg(BT_HI1, np.concatenate([b_hh1[512:], b_ih1[512:]]))
    biasT = biasT.astype(BF16)

    oh8 = np.zeros((8, 512), f4)
    for k in range(8):
        oh8[k, (k % 4) * 128 : (k % 4 + 1) * 128] = 1.0
    oh8 = oh8.astype(BF16)

    brows = np.zeros(NROW * 128, f4)
    brows[_BOUT * 128 : _BOUT * 128 + 256] = b_out
    brows = brows.astype(BF16)[None, :]

    shared = dict(wrz0=wrz0, win0=win0, whn0=whn0, wrz1=wrz1,
                  win1=win1, whn1=whn1, wout=wout,
                  biasT=biasT, oh8=oh8, brows=brows)
    in_maps = []
    for i in range(N_CORES):
        m = dict(shared)
        m["enc"] = enc_last[i * PC : (i + 1) * PC]
        in_maps.append(m)
    return in_maps


_CACHE = {}


def _run(in_maps, trace=False):
    from concourse.bass_utils import run_bass_kernel_spmd

    if "nc" not in _CACHE:
        _CACHE["nc"] = build_kernel()
    nc = _CACHE["nc"]
    res = run_bass_kernel_spmd(
        nc, in_maps, core_ids=list(range(N_CORES)), trace=trace
    )
    preds = np.concatenate([res.results[i]["out"] for i in range(N_CORES)],
                       axis=0).astype(np.float32)
    return preds, res


def kernel(encoded_features, step_emb, W_ih0, W_hh0, b_ih0, b_hh0,
           W_ih1, W_hh1, b_ih1, b_hh1, W_out, b_out, num_steps):
    assert int(num_steps) == STEPS
    in_maps = _prep_inputs(encoded_features, step_emb, W_ih0, W_hh0, b_ih0,
                           b_hh0, W_ih1, W_hh1, b_ih1, b_hh1, W_out, b_out)
    preds, _ = _run(in_maps, trace=False)
    return preds



# revision 48
# speedup vs baseline: 1.0368x; 1.0031x over previous
"""Trainium2 Bass kernel for the autoregressive GRU decode head.

Problem: context = mean over zones of encoded_features[:, -1]  -> (B, D)
then 12 autoregressive steps of a 2-layer GRU (H=256) + linear projection
to N=256 zones.  B=1024, data-parallel across 8 NeuronCores (128 batch each).

Structure (per core, feature-major / "transposed" activations):
  actT (128p, 4 slots, 128) bf16 : [h0 c0, h0 c1, h1 c0, h1 c1]
       slot holds h[b, c*128 + p] at [p, b]    (c = chunk of the 256-dim)
  Gate tensors (PSUM) use layout [p, c*128 + b].
  Matmuls: out(gate_chunk, B) = lhsT.T @ rhs, lhsT = W^T tile (K<=128, M=128),
  rhs = actT slot (K=128, B=128), K-chunks accumulated in PSUM.
  The prediction feedback is algebraically folded into layer 0's weights:
  W_pred @ (W_out h1 + b_out) = (W_pred W_out) h1 + W_pred b_out.
  Chain per layer (PSUM g_rz = [r|z], g_hi = [ghn|gin]; all gate biases
  are pre-accumulated into the PSUM banks by one K=8 matmul per bank
  against a one-hot moving operand -- a start=True matmul resets the
  whole bank, so each bank carries exactly one accumulation group):
    r = sigmoid(g_rz[0:256])     single 256-wide ACT
    z = sigmoid(g_rz[256:512])   single 256-wide ACT
    hnb = ghn (DVE copy to SBUF bf16, prefetched in DVE idle)
    t = hnb * r                  TT 256w (2x mode)
    v = gin + t                  STT 256w from PSUM
    n = tanh(v); zm1 = 1-z (off-path); c = z*h; q = zm1*n; h' = c + q
  Critical-path trick: the NEXT matmul block needs W @ h' with
  h' = c + (1-z)*n, so it is issued as W @ c + W @ q against the c/q
  tiles, which exist before h' does -- the r-gate matmuls of the next
  layer/step fire right after the chain instead of serializing behind
  h' (and no negated weight copies are needed).  The z / in-gate
  matmuls use h' directly (they are needed later).  All h1(t-1)-only
  work runs during chain0; all h0'(t)-only work (incl. step t+1's
  rz0/hn0 h-parts and biases) during chain1; wout(t-1) matmuls during
  chain0, its PSUM->SBUF copy at the end of chain1 so the ACT COPY
  queues behind the critical tanh.
The encoded_features slice streams as fp8 in 9 chunks split across BOTH
HWDGE queues (even->sync, odd->scalar) so the per-chunk completion
overheads overlap; weights follow on the scalar queue ordered by first
use.  The zone mean splits between a DVE pairwise TT-add tree (chunks
1,3) and PE identity-matmul PSUM accumulation (the rest) -- the PE owns
the first chunks so it starts at first arrival and stays continuously
busy (idle gaps downclock the tensor engine 2-4x).
"""

import sys

for _p in ("/opt/trn_rl_repo",):
    if _p not in sys.path:
        sys.path.insert(0, _p)

import numpy as np
import ml_dtypes

import concourse.bass as bass
import concourse.tile as tile
from concourse import mybir
from concourse.vector_clock import ScopedClock

BF16 = ml_dtypes.bfloat16

B, T, NZ, D = 1024, 8, 256, 256
H = 256
STEPS = 12
N_CORES = 8
PC = B // N_CORES  # 128 batch per core

F32 = mybir.dt.float32
BF = mybir.dt.bfloat16
F8 = mybir.dt.float8e4
AF = mybir.ActivationFunctionType
OP = mybir.AluOpType

# biasT (bf16, [8, NGRP*128]): per-group bias stacks for the K=8 bias
# matmuls (one per PSUM bank+step: a start=True matmul resets the WHOLE
# bank, so each bank gets exactly one group whose first matmul adds all
# four 128-chunk biases).  Group g's lhsT is biasT[0:8, g*128:(g+1)*128];
# rows 0:4 hold bf16-hi chunk biases, rows 4:8 the bf16-lo residuals.
BT_RZ0 = 0                    # 12 steps: [r c0, r c1, z c0, z c1] of L0
BT_RZ1 = BT_RZ0 + STEPS
BT_HI0 = BT_RZ1 + 1           # 12 steps: [hn c0, hn c1, in c0, in c1]
BT_HI1 = BT_HI0 + STEPS
NGRP = BT_HI1 + 1

# brow (bf16, [1, 2*128]): b_out as a moving row for the batch-major wout
_BOUT = 0
NROW = 2


def _install_tile_drain_patch():
    """walrus (CoreV3) rejects >1 sync wait on the tail drain; spill extras
    onto preceding sync nops."""
    if getattr(tile.TileContext, "_drain_patch_installed", False):
        return

    def _patched(self, tick_clock, wait_clock):
        nc = self.nc
        bb = nc.cur_bb.bb
        drain_bi = nc.sync.drain()
        drain_inst = drain_bi.ins
        wait_clock.add_sem_waits(
            drain_inst, ScopedClock({None: tick_clock.global_clock})
        )
        w = drain_inst.sync_info.on_wait if drain_inst.sync_info else None
        maxw = 1
        if w and len(w) > maxw:
            extra = list(w[maxw:])
            drain_inst.sync_info.on_wait = list(w[:maxw])
            idx = bb.instructions.index(drain_inst)
            nops = []
            for i in range(0, len(extra), maxw):
                nop_bi = nc.sync.nop()
                nop = nop_bi.ins
                si = nop.sync_info
                nop.sync_info = mybir.SyncInfo(
                    on_wait=extra[i : i + maxw],
                    on_update=(si.on_update if si else []),
                )
                bb.instructions.remove(nop)
                nops.append(nop)
            bb.instructions[idx:idx] = nops
        nc.all_engine_barrier()
        popped = nc._tile_sem_poison_stack.pop()
        assert popped is self._sem_poison
        nc.clear_and_free_semaphores(list(self.sems.allocated().values()))
        nc.all_engine_barrier()

    tile.TileContext._drain_and_barrier = _patched
    tile.TileContext._drain_patch_installed = True


def _split_waits(nc, maxw=1):
    """This walrus build rejects instructions carrying more than ~1 sem
    wait; spill extra waits onto same-engine nops placed just before."""
    for bb in nc.main_func.blocks:
        new_list = []
        for inst in bb.instructions:
            si = inst.sync_info
            w = list(si.on_wait) if si and si.on_wait else []
            if len(w) > maxw:
                keep = w[len(w) - maxw:]
                extra = w[: len(w) - maxw]
                si.on_wait = keep
                for i in range(0, len(extra), maxw):
                    nop = mybir.InstNoOp(
                        name=f"{inst.name}-sw{i}", ins=[], outs=[]
                    )
                    nop.engine = inst.engine
                    nop.sync_info = mybir.SyncInfo(
                        on_wait=extra[i : i + maxw], on_update=[]
                    )
                    nc.register_instruction(nop)
                    new_list.append(nop)
            new_list.append(inst)
        bb.instructions[:] = new_list


class _Group:
    """Tracks start/stop flags for a PSUM accumulation group whose matmuls
    are emitted in several program-order batches."""

    def __init__(self, total):
        self.total = total
        self.emitted = 0

    def flags(self):
        start = self.emitted == 0
        self.emitted += 1
        return start, self.emitted == self.total


FILL_CQ = 0     # junk matmuls before each layer.s c/q-dependent block
FILL_P1 = 0      # before each phase-1 PE zone-sum chunk group
FILL_RAMP = 0   # bridging the phase-1 -> decode transition


def build_kernel(nsteps=12):
    """Build the per-core Bass graph (SPMD: same graph on all 8 cores)."""
    _install_tile_drain_patch()
    nc = bass.Bass()

    enc = nc.declare_dram_parameter("enc", [PC, NZ, D], F8, isOutput=False)
    wrz0 = nc.declare_dram_parameter("wrz0", [4, 128, 512], BF, isOutput=False)
    win0 = nc.declare_dram_parameter("win0", [2, 128, 256], BF, isOutput=False)
    whn0 = nc.declare_dram_parameter("whn0", [2, 128, 256], BF, isOutput=False)
    wrz1 = nc.declare_dram_parameter("wrz1", [4, 128, 512], BF, isOutput=False)
    win1 = nc.declare_dram_parameter("win1", [2, 128, 256], BF, isOutput=False)
    whn1 = nc.declare_dram_parameter("whn1", [2, 128, 256], BF, isOutput=False)
    wout = nc.declare_dram_parameter("wout", [2, 128, 256], BF, isOutput=False)
    biasT = nc.declare_dram_parameter("biasT", [8, NGRP * 128], BF,
                                      isOutput=False)
    oh8 = nc.declare_dram_parameter("oh8", [8, 512], BF, isOutput=False)
    brows = nc.declare_dram_parameter("brows", [1, NROW * 128], BF,
                                      isOutput=False)
    out = nc.declare_dram_parameter("out", [PC, STEPS, NZ], BF, isOutput=True)

    with tile.TileContext(nc) as tc:
        with (
            tc.tile_pool(name="consts", bufs=1) as consts,
            tc.tile_pool(name="state", bufs=1) as state,
            tc.tile_pool(name="enc_pool", bufs=4) as enc_pool,
            tc.tile_pool(name="gates", bufs=2) as gates,
            tc.tile_pool(name="ostage", bufs=2) as ostage,
            tc.tile_pool(name="psum", bufs=1, space="PSUM") as psum,
        ):
            # ---- phase 1 DMA: enc all on the sync queue, in tree order ----
            # PE consumes ~2x faster than the DVE tree: it owns the first
            # chunks plus the tail; the DVE takes two early-arriving chunks
            ZCHS = [32, 32, 32, 32, 32, 32, 32, 16, 16]
            PE_CHUNKS = (0, 2, 4, 5, 6, 7, 8)
            NCH = len(ZCHS)
            # enc split across both HWDGE queues (even->sync, odd->scalar)
            # so the two queues' per-chunk completion overheads overlap and
            # the aggregate stream runs at the DMA bandwidth roofline
            e_tiles = []
            z0 = 0
            for i, zch in enumerate(ZCHS):
                e_sb = enc_pool.tile([128, 32 * D], F8, tag="echunk",
                                     bufs=13)
                eng = nc.sync if i % 2 == 0 else nc.scalar
                eng.dma_start(e_sb[:, : zch * D], enc[:, z0 : z0 + zch, :])
                e_tiles.append(e_sb)
                z0 += zch

            # small constants then weights behind the enc stream, ordered
            # by first use
            biasT_sb = consts.tile([8, NGRP * 128], BF, tag="biasT")
            nc.scalar.dma_start(biasT_sb[:], biasT[:])
            oh8_sb = consts.tile([8, 512], BF, tag="oh8")
            nc.scalar.dma_start(oh8_sb[:], oh8[:])
            brow_sb = consts.tile([1, NROW * 128], BF, tag="brow")
            nc.scalar.dma_start(brow_sb[:], brows[:])
            w_sb = {}
            for name, ap, kc, mdim in (
                ("wrz0", wrz0, 4, 512),
                ("whn0", whn0, 2, 256),
                ("wrz1", wrz1, 4, 512),
                ("whn1", whn1, 2, 256),
                ("win1", win1, 2, 256),
                ("wout", wout, 2, 256),
                ("win0", win0, 2, 256),
            ):
                t_ = consts.tile([128, kc, mdim], BF, tag=name)
                nc.scalar.dma_start(t_[:], ap.rearrange("k p m -> p k m"))
                w_sb[name] = t_

            ones_row = consts.tile([1, 128], BF, tag="ones")
            nc.gpsimd.memset(ones_row[:], 1.0)
            identity = consts.tile([128, 128], F32, tag="ident")
            nc.gpsimd.memset(identity[:], 0.0)
            nc.gpsimd.affine_select(
                out=identity[:],
                in_=identity[:],
                compare_op=OP.not_equal,
                fill=1.0,
                base=0,
                pattern=[[-1, 128]],
                channel_multiplier=1,
            )
            ident_f8 = consts.tile([128, 128], F8, tag="identf8")
            nc.gpsimd.memset(ident_f8[:], 0.0)
            nc.gpsimd.affine_select(
                out=ident_f8[:],
                in_=ident_f8[:],
                compare_op=OP.not_equal,
                fill=1.0,
                base=0,
                pattern=[[-1, 128]],
                channel_multiplier=1,
            )
            # prewarm the sigmoid/tanh ACT table during phase 1
            warm = consts.tile([128, 1], F32, tag="warm")
            nc.scalar.activation(warm[:], identity[:, 0:1], AF.Sigmoid)
            gwarm = consts.tile([128, 128], BF, tag="gwarm")
            nc.gpsimd.tensor_tensor(gwarm[:], ident_f8[:], ident_f8[:],
                                    OP.add)

            # PE keepalive: junk matmuls emitted at known stall points keep
            # the tensor engine's p-state at max (idle gaps downclock it and
            # the next real matmuls run 2-4x slower).
            jk_ps = psum.tile([128, 512], F32, tag="junk", bufs=1)

            def fill(n):
                for _ in range(n):
                    nc.tensor.matmul(jk_ps[:, 0:64], gwarm[:],
                                     gwarm[:, 0:64], start=True, stop=True)

            def fill_t(tsrc, n):
                # junk matmuls reading a chain tile: become ready when the
                # chain op lands, bridging PE idle before the next real
                # matmul group (p-state keepalive)
                for _ in range(n):
                    nc.tensor.matmul(jk_ps[:, 0:256], gwarm[:],
                                     tsrc[:, 0:256], start=True, stop=True)

            def fill_e(e_sb, n):
                # junk matmuls whose moving operand is a just-arrived enc
                # chunk: they become ready exactly when the chunk lands,
                # bridging PE idle between chunk arrivals (p-state keepalive)
                for _ in range(n):
                    nc.tensor.matmul(jk_ps[:], ident_f8[:],
                                     e_sb[:, 0:512], start=True, stop=True)

            # ---- phase 1: zone-mean; DVE pairwise tree for most chunks,
            # PE identity-matmul accumulation for PE_CHUNKS (the PE is
            # otherwise idle during the stream; PSUM accumulates in f32)
            tmpf = state.tile([128, 512], F32, tag="tmpf")
            ptl = state.tile([128, 256], F32, tag="ptl")
            acc = state.tile([128, 256], F32, tag="acc")
            zsum = psum.tile([128, 256], F32, tag="outp", bufs=2)
            n_pe = sum(ZCHS[i] for i in PE_CHUNKS)
            pe_grp = _Group(n_pe)
            first_dve = True
            for i in range(NCH):
                e_sb = e_tiles[i]
                if i in PE_CHUNKS:
                    for z in range(ZCHS[i]):
                        st, sp = pe_grp.flags()
                        nc.tensor.matmul(
                            zsum[:], ident_f8[:],
                            e_sb[:, z * D : (z + 1) * D],
                            start=st, stop=sp,
                        )
                    continue
                w = ZCHS[i] * D
                scr = state.tile([128, 16 * D], BF, tag="scr")
                h = w // 2
                nc.vector.tensor_tensor(
                    scr[:, 0:h], e_sb[:, 0:h], e_sb[:, h:w], OP.add
                )
                w = h
                while w > 4 * D:
                    h = w // 2
                    nc.vector.tensor_tensor(
                        scr[:, 0:h], scr[:, 0:h], scr[:, h:w], OP.add
                    )
                    w = h
                nc.vector.tensor_tensor(
                    tmpf[:], scr[:, 0 : 2 * D], scr[:, 2 * D : 4 * D], OP.add
                )
                if first_dve:
                    nc.vector.tensor_tensor(
                        acc[:], tmpf[:, 0:D], tmpf[:, D : 2 * D], OP.add
                    )
                    first_dve = False
                else:
                    nc.vector.tensor_tensor(
                        ptl[:], tmpf[:, 0:D], tmpf[:, D : 2 * D], OP.add
                    )
                    nc.vector.tensor_tensor(acc[:], acc[:], ptl[:], OP.add)
            ztot = state.tile([128, 256], F32, tag="ztot")
            nc.scalar.activation(ztot[:], zsum[:], AF.Copy)

            # ---- state: actT slots [h0c0, h0c1, h1c0, h1c1] ----
            # the acc + ztot merge rides the PSUM accumulation of the
            # two transposes
            actT = state.tile([128, 4, 128], BF, tag="actT")
            for c in range(2):
                cs = slice(c * 128, (c + 1) * 128)
                ctps = psum.tile([128, 128], F32, tag="outp", bufs=2)
                nc.tensor.matmul(ctps[:], acc[:, cs], identity[:],
                                 is_transpose=True, start=True, stop=False)
                nc.tensor.matmul(ctps[:], ztot[:, cs], identity[:],
                                 is_transpose=True, start=False, stop=True)
                nc.scalar.activation(actT[:, c, :], ctps[:], AF.Copy,
                                     scale=1.0 / NZ)
                nc.scalar.activation(actT[:, 2 + c, :], ctps[:], AF.Copy,
                                     scale=1.0 / NZ)

            # ---- decode-phase emitters ----
            def gate_mms(g, grp, w_t, kis, slots, mlo, mhi, coloff=0):
                """slots entries: int -> actT slot; (tile, k) -> gates tile
                chunk k used as the moving operand."""
                for m in range(mlo, mhi):
                    ms = slice((coloff + m) * 128, (coloff + m + 1) * 128)
                    wms = slice(m * 128, (m + 1) * 128)
                    for ki, slot in zip(kis, slots):
                        if isinstance(slot, tuple):
                            src, k = slot
                            rhs = src[:, k * 128 : (k + 1) * 128]
                        else:
                            rhs = actT[:, slot, :]
                        st, sp = grp.flags()
                        nc.tensor.matmul(
                            g[:, ms], w_t[:, ki, wms], rhs, start=st, stop=sp,
                        )

            # ---- phase 2: 12 decode steps ----
            cur = {}

            def bias_mm(g, grp, gidx):
                """Start a bank's single accumulation group by adding all
                four 128-chunk biases via one K=8 matmul against a one-hot
                moving operand (rows 0:4 bf16-hi, 4:8 bf16-lo)."""
                st, sp = grp.flags()
                nc.tensor.matmul(
                    g[:, 0:512], biasT_sb[0:8, gidx * 128 : (gidx + 1) * 128],
                    oh8_sb[0:8, 0:512], start=st, stop=sp,
                )

            def emit_pre0(t):
                """rz0-hh / hn0 / biases for step t: depend only on
                h0(t-1)."""
                g_rz0 = psum.tile([128, 512], F32, tag="rz0", bufs=2)
                g_hi0 = psum.tile([128, 512], F32, tag="hi0", bufs=1)
                grz0 = _Group(1 + 8 + (12 if t > 0 else 0))
                ghi0 = _Group(1 + 4 + (4 if t > 0 else 0))
                bias_mm(g_rz0, grz0, BT_RZ0 + t)
                bias_mm(g_hi0, ghi0, BT_HI0 + t)
                gate_mms(g_rz0, grz0, w_sb["wrz0"], (2, 3), (0, 1), 0, 4)
                gate_mms(g_hi0, ghi0, w_sb["whn0"], (0, 1), (0, 1), 0, 2)
                cur[t] = (g_rz0, g_hi0, grz0, ghi0)

            fill(FILL_RAMP)
            emit_pre0(0)
            prev_w = None
            prev_cq = None   # (c_, q_) of the most recent layer-1 chain

            def emit_wout_mms():
                """wout matmuls on h1 (read actT slots 2,3 -- must be
                emitted before layer-1 overwrites them)."""
                g_pb = psum.tile([128, 256], F32, tag="outp", bufs=2)
                gout = _Group(3)
                st, sp = gout.flags()
                nc.tensor.matmul(
                    g_pb[:], ones_row[:],
                    brow_sb[0:1, _BOUT * 128 : (_BOUT + 2) * 128],
                    start=st, stop=sp,
                )
                for ki, slot in ((0, 2), (1, 3)):
                    st, sp = gout.flags()
                    nc.tensor.matmul(
                        g_pb[:], actT[:, slot, :], w_sb["wout"][:, ki, :],
                        start=st, stop=sp,
                    )
                return g_pb

            def emit_wout_copy(g_pb, t):
                """PSUM->SBUF stage + output DMA; emitted at the end of the
                layer-1 block so the ACT COPY queues after the critical
                tanh, not before it."""
                o_ = ostage.tile([128, 256], BF, tag="ost")
                nc.scalar.activation(o_[:], g_pb[:], AF.Copy)
                nc.sync.dma_start(out[:, t, :], o_[:])

            for t in range(nsteps):
                g_rz0, g_hi0, grz0, ghi0 = cur.pop(t)
                g_rz1 = psum.tile([128, 512], F32, tag="rz1", bufs=1)
                g_hi1 = psum.tile([128, 512], F32, tag="hi1", bufs=1)
                grz1 = _Group(1 + 8 + 12)
                ghi1 = _Group(1 + 4 + 4)

                for layer in range(2):
                    if layer == 0:
                        g_rz, g_hi, grz, ghi = g_rz0, g_hi0, grz0, ghi0
                        h_sl, x_sl = 0, (2, 3)      # h slots; x = other h
                        w_f, w_in = w_sb["wrz0"], w_sb["win0"]
                        cq = prev_cq if t > 0 else None
                    else:
                        g_rz, g_hi, grz, ghi = g_rz1, g_hi1, grz1, ghi1
                        h_sl, x_sl = 2, (0, 1)
                        w_f, w_in = w_sb["wrz1"], w_sb["win1"]
                        cq = this_cq  # layer-0 chain of this step

                    # r-part of the input block via W@c + (-W)@q: fires as
                    # soon as the previous chain's c/q exist (before h');
                    # in-gate mms right behind so gin is in PSUM before v_
                    s_ = gates.tile([128, 512], BF, tag=f"s{layer}")
                    hnb = gates.tile([128, 256], BF, tag=f"hb{layer}")
                    if cq is not None:
                        c_p, q_p = cq
                        fill(FILL_CQ)
                        gate_mms(g_rz, grz, w_f, (0, 1),
                                 ((c_p, 0), (c_p, 1)), 0, 2)
                        gate_mms(g_rz, grz, w_f, (0, 1),
                                 ((q_p, 0), (q_p, 1)), 0, 2)
                    # stage the hn gate to SBUF bf16 on the DVE; emitted
                    # BEFORE the in-mms so its (tile-granular) g_hi dep
                    # closes at the long-done hn mms -> runs in DVE idle
                    nc.vector.tensor_scalar_add(hnb[:], g_hi[:, 0:256], 0.0)
                    if cq is not None:
                        gate_mms(g_hi, ghi, w_in, (0, 1), x_sl, 0, 2,
                                 coloff=2)
                    # biases pre-accumulated into PSUM: single 256-wide ACTs
                    nc.scalar.activation(s_[:, 0:256], g_rz[:, 0:256],
                                         AF.Sigmoid)
                    # z-part plainly on h' of the previous chain
                    if cq is not None:
                        gate_mms(g_rz, grz, w_f, (0, 1), x_sl, 2, 4)
                    nc.scalar.activation(s_[:, 256:512], g_rz[:, 256:512],
                                         AF.Sigmoid)
                    t_ = gates.tile([128, 256], BF, tag=f"t{layer}")
                    nc.vector.tensor_tensor(t_[:], hnb[:], s_[:, 0:256],
                                            OP.mult)
                    if layer == 0:
                        # layer-1 parts that need only h1(t-1)
                        bias_mm(g_rz1, grz1, BT_RZ1)
                        bias_mm(g_hi1, ghi1, BT_HI1)
                        gate_mms(g_rz1, grz1, w_sb["wrz1"], (2, 3), (2, 3),
                                 0, 4)
                        gate_mms(g_hi1, ghi1, w_sb["whn1"], (0, 1), (2, 3),
                                 0, 2)
                    else:
                        # step t+1 parts that need only h0'(t)
                        if t + 1 < nsteps:
                            emit_pre0(t + 1)
                    v_ = gates.tile([128, 256], BF, tag=f"v{layer}")
                    nc.vector.scalar_tensor_tensor(
                        v_[:], g_hi[:, 256:512], 0.0, t_[:],
                        op0=OP.add, op1=OP.add,
                    )
                    # 256-wide tail: n/c/zm1/q/h'.  q = (z-1)*n as a 2x TT
                    # against the precomputed zm1 (the STT form runs 1x).
                    c_ = gates.tile([128, 256], BF, tag=f"c{layer}")
                    n_ = gates.tile([128, 256], BF, tag=f"n{layer}")
                    q_ = gates.tile([128, 256], BF, tag=f"q{layer}")
                    zm1 = gates.tile([128, 256], BF, tag=f"z{layer}")
                    nc.scalar.activation(n_[:], v_[:], AF.Tanh)
                    nc.vector.tensor_tensor(
                        c_[:], s_[:, 256:512], actT[:, h_sl : h_sl + 2, :],
                        OP.mult,
                    )
                    nc.vector.tensor_scalar(zm1[:], s_[:, 256:512], 1.0,
                                            -1.0, op0=OP.subtract,
                                            op1=OP.mult)
                    nc.vector.tensor_tensor(q_[:], zm1[:], n_[:], OP.mult)
                    nc.vector.tensor_tensor(
                        actT[:, h_sl : h_sl + 2, :], c_[:], q_[:],
                        OP.add,
                    )

                    if layer == 0:
                        this_cq = (c_, q_)
                        if prev_w is not None:
                            prev_pb = emit_wout_mms()
                    else:
                        prev_cq = (c_, q_)
                        if prev_w is not None:
                            emit_wout_copy(prev_pb, prev_w)
                prev_w = t
            g_pb = emit_wout_mms()
            emit_wout_copy(g_pb, prev_w)

    _split_waits(nc)
    return nc


def _prep_inputs(encoded_features, step_emb, W_ih0, W_hh0, b_ih0, b_hh0,
                 W_ih1, W_hh1, b_ih1, b_hh1, W_out, b_out):
    """Host-side: slice/shard the big input, transpose + cast weights,
    fold the output projection into layer-0 input weights, fold the
    step-embedding matmul + all additive constants into bias columns."""
    f4 = np.float32
    enc_last = np.asarray(encoded_features)[:, -1].astype(ml_dtypes.float8_e4m3)
    enc_last = np.ascontiguousarray(enc_last)

    W_ih0 = np.asarray(W_ih0, f4)
    W_hh0 = np.asarray(W_hh0, f4)
    W_ih1 = np.asarray(W_ih1, f4)
    W_hh1 = np.asarray(W_hh1, f4)
    W_out = np.asarray(W_out, f4)
    step_emb = np.asarray(step_emb, f4)
    b_ih0 = np.asarray(b_ih0, f4)
    b_hh0 = np.asarray(b_hh0, f4)
    b_ih1 = np.asarray(b_ih1, f4)
    b_hh1 = np.asarray(b_hh1, f4)
    b_out = np.asarray(b_out, f4)

    W_emb = W_ih0[:, :D]          # (768, 256)
    W_pred = W_ih0[:, D:]         # (768, 256)
    W_fold = W_pred @ W_out       # (768, 256): pred feedback folded onto h1
    b_fold = W_pred @ b_out       # (768,)

    # gi_emb[t] = W_emb @ step_emb[t] + b_ih0  -> (12, 768)
    gi_emb = step_emb[:STEPS] @ W_emb.T + b_ih0[None, :]

    def kstack(*mats_cols):
        chunks = []
        for mat, cols in mats_cols:
            mt = np.ascontiguousarray(mat.T[:, cols])  # (K, M)
            for k in range(0, mt.shape[0], 128):
                chunks.append(mt[k : k + 128])
        return np.stack(chunks).astype(BF16)  # (nk, 128, M)

    rz = slice(0, 512)
    ng = slice(512, 768)
    wrz0 = kstack((W_fold, rz), (W_hh0, rz))          # K: h1c0,h1c1,h0c0,h0c1
    win0 = kstack((W_fold, ng))
    whn0 = kstack((W_hh0, ng))
    wrz1 = kstack((W_ih1, rz), (W_hh1, rz))           # K: h0c0,h0c1,h1c0,h1c1
    win1 = kstack((W_ih1, ng))
    whn1 = kstack((W_hh1, ng))
    wout = np.stack([np.ascontiguousarray(W_out.T)[k : k + 128] for k in (0, 128)]
                    ).astype(BF16)                    # (2, 128, 256)

    biasT = np.zeros((8, NGRP * 128), f4)

    def putg(g, vec):
        # bf16 hi/lo split: rows 0:nk hold bf16(vec) chunks, nk:2nk residual
        nk = len(vec) // 128
        hi = vec.astype(BF16).astype(f4)
        lo = vec - hi
        for k in range(nk):
            biasT[k, g * 128 : (g + 1) * 128] = hi[k * 128 : (k + 1) * 128]
            biasT[nk + k, g * 128 : (g + 1) * 128] = (
                lo[k * 128 : (k + 1) * 128])

    for t in range(STEPS):
        extra = b_fold if t > 0 else np.zeros_like(b_fold)
        putg(BT_RZ0 + t, gi_emb[t, :512] + b_hh0[:512] + extra[:512])
        putg(BT_HI0 + t, np.concatenate(
            [b_hh0[512:], gi_emb[t, 512:] + extra[512:]]))
    putg(BT_RZ1, b_ih1[:512] + b_hh1[:512])
    putg(BT_HI1, np.concatenate([b_hh1[512:], b_ih1[512:]]))
    biasT = biasT.astype(BF16)

    oh8 = np.zeros((8, 512), f4)
    for k in range(8):
        oh8[k, (k % 4) * 128 : (k % 4 + 1) * 128] = 1.0
    oh8 = oh8.astype(BF16)

    brows = np.zeros(NROW * 128, f4)
    brows[_BOUT * 128 : _BOUT * 128 + 256] = b_out
    brows = brows.astype(BF16)[None, :]

    shared = dict(wrz0=wrz0, win0=win0, whn0=whn0, wrz1=wrz1,
                  win1=win1, whn1=whn1, wout=wout,
                  biasT=biasT, oh8=oh8, brows=brows)
    in_maps = []
    for i in range(N_CORES):
        m = dict(shared)
        m["enc"] = enc_last[i * PC : (i + 1) * PC]
        in_maps.append(m)
    return in_maps


_CACHE = {}


def _run(in_maps, trace=False):
    from concourse.bass_utils import run_bass_kernel_spmd

    if "nc" not in _CACHE:
        _CACHE["nc"] = build_kernel()
    nc = _CACHE["nc"]
    res = run_bass_kernel_spmd(
        nc, in_maps, core_ids=list(range(N_CORES)), trace=trace
    )
    preds = np.concatenate([res.results[i]["out"] for i in range(N_CORES)],
                       axis=0).astype(np.float32)
    return preds, res


def kernel(encoded_features, step_emb, W_ih0, W_hh0, b_ih0, b_hh0,
           W_ih1, W_hh1, b_ih1, b_hh1, W_out, b_out, num_steps):
    assert int(num_steps) == STEPS
    in_maps = _prep_inputs(encoded_features, step_emb, W_ih0, W_hh0, b_ih0,
                           b_hh0, W_ih1, W_hh1, b_ih1, b_hh1, W_out, b_out)
    preds, _ = _run(in_maps, trace=False)
    return preds



# revision 49
# speedup vs baseline: 1.0448x; 1.0077x over previous
"""Trainium2 Bass kernel for the autoregressive GRU decode head.

Problem: context = mean over zones of encoded_features[:, -1]  -> (B, D)
then 12 autoregressive steps of a 2-layer GRU (H=256) + linear projection
to N=256 zones.  B=1024, data-parallel across 8 NeuronCores (128 batch each).

Structure (per core, feature-major / "transposed" activations):
  actT (128p, 4 slots, 128) bf16 : [h0 c0, h0 c1, h1 c0, h1 c1]
       slot holds h[b, c*128 + p] at [p, b]    (c = chunk of the 256-dim)
  Gate tensors (PSUM) use layout [p, c*128 + b].
  Matmuls: out(gate_chunk, B) = lhsT.T @ rhs, lhsT = W^T tile (K<=128, M=128),
  rhs = actT slot (K=128, B=128), K-chunks accumulated in PSUM.
  The prediction feedback is algebraically folded into layer 0's weights:
  W_pred @ (W_out h1 + b_out) = (W_pred W_out) h1 + W_pred b_out.
  Chain per layer (PSUM g_rz = [r|z], g_hi = [ghn|gin]; all gate biases
  are pre-accumulated into the PSUM banks by one K=8 matmul per bank
  against a one-hot moving operand -- a start=True matmul resets the
  whole bank, so each bank carries exactly one accumulation group):
    r = sigmoid(g_rz[0:256])     single 256-wide ACT
    z = sigmoid(g_rz[256:512])   single 256-wide ACT
    hnb = ghn (DVE copy to SBUF bf16, prefetched in DVE idle)
    t = hnb * r                  TT 256w (2x mode)
    v = gin + t                  STT 256w from PSUM
    n = tanh(v); zm1 = 1-z (off-path); c = z*h; q = zm1*n; h' = c + q
  Critical-path trick: the NEXT matmul block needs W @ h' with
  h' = c + (1-z)*n, so it is issued as W @ c + W @ q against the c/q
  tiles, which exist before h' does -- the r-gate matmuls of the next
  layer/step fire right after the chain instead of serializing behind
  h' (and no negated weight copies are needed).  The z / in-gate
  matmuls use h' directly (they are needed later).  All h1(t-1)-only
  work runs during chain0; all h0'(t)-only work (incl. step t+1's
  rz0/hn0 h-parts and biases) during chain1; wout(t-1) matmuls during
  chain0, its PSUM->SBUF copy at the end of chain1 so the ACT COPY
  queues behind the critical tanh.
The encoded_features slice streams as fp8 in 9 chunks split across BOTH
HWDGE queues (even->sync, odd->scalar) so the per-chunk completion
overheads overlap; weights follow on the scalar queue ordered by first
use.  The zone mean splits between a DVE pairwise TT-add tree (chunks
1,3) and PE identity-matmul PSUM accumulation (the rest) -- the PE owns
the first chunks so it starts at first arrival and stays continuously
busy (idle gaps downclock the tensor engine 2-4x).
"""

import sys

for _p in ("/opt/trn_rl_repo",):
    if _p not in sys.path:
        sys.path.insert(0, _p)

import numpy as np
import ml_dtypes

import concourse.bass as bass
import concourse.tile as tile
from concourse import mybir
from concourse.vector_clock import ScopedClock

BF16 = ml_dtypes.bfloat16

B, T, NZ, D = 1024, 8, 256, 256
H = 256
STEPS = 12
N_CORES = 8
PC = B // N_CORES  # 128 batch per core

F32 = mybir.dt.float32
BF = mybir.dt.bfloat16
F8 = mybir.dt.float8e4
AF = mybir.ActivationFunctionType
OP = mybir.AluOpType

# biasT (bf16, [8, NGRP*128]): per-group bias stacks for the K=8 bias
# matmuls (one per PSUM bank+step: a start=True matmul resets the WHOLE
# bank, so each bank gets exactly one group whose first matmul adds all
# four 128-chunk biases).  Group g's lhsT is biasT[0:8, g*128:(g+1)*128];
# rows 0:4 hold bf16-hi chunk biases, rows 4:8 the bf16-lo residuals.
BT_RZ0 = 0                    # 12 steps: [r c0, r c1, z c0, z c1] of L0
BT_RZ1 = BT_RZ0 + STEPS
BT_HI0 = BT_RZ1 + 1           # 12 steps: [hn c0, hn c1, in c0, in c1]
BT_HI1 = BT_HI0 + STEPS
NGRP = BT_HI1 + 1

# brow (bf16, [1, 2*128]): b_out as a moving row for the batch-major wout
_BOUT = 0
NROW = 2


def _install_tile_drain_patch():
    """walrus (CoreV3) rejects >1 sync wait on the tail drain; spill extras
    onto preceding sync nops."""
    if getattr(tile.TileContext, "_drain_patch_installed", False):
        return

    def _patched(self, tick_clock, wait_clock):
        nc = self.nc
        bb = nc.cur_bb.bb
        drain_bi = nc.sync.drain()
        drain_inst = drain_bi.ins
        wait_clock.add_sem_waits(
            drain_inst, ScopedClock({None: tick_clock.global_clock})
        )
        w = drain_inst.sync_info.on_wait if drain_inst.sync_info else None
        maxw = 1
        if w and len(w) > maxw:
            extra = list(w[maxw:])
            drain_inst.sync_info.on_wait = list(w[:maxw])
            idx = bb.instructions.index(drain_inst)
            nops = []
            for i in range(0, len(extra), maxw):
                nop_bi = nc.sync.nop()
                nop = nop_bi.ins
                si = nop.sync_info
                nop.sync_info = mybir.SyncInfo(
                    on_wait=extra[i : i + maxw],
                    on_update=(si.on_update if si else []),
                )
                bb.instructions.remove(nop)
                nops.append(nop)
            bb.instructions[idx:idx] = nops
        nc.all_engine_barrier()
        popped = nc._tile_sem_poison_stack.pop()
        assert popped is self._sem_poison
        nc.clear_and_free_semaphores(list(self.sems.allocated().values()))
        nc.all_engine_barrier()

    tile.TileContext._drain_and_barrier = _patched
    tile.TileContext._drain_patch_installed = True


def _split_waits(nc, maxw=1):
    """This walrus build rejects instructions carrying more than ~1 sem
    wait; spill extra waits onto same-engine nops placed just before."""
    for bb in nc.main_func.blocks:
        new_list = []
        for inst in bb.instructions:
            si = inst.sync_info
            w = list(si.on_wait) if si and si.on_wait else []
            if len(w) > maxw:
                keep = w[len(w) - maxw:]
                extra = w[: len(w) - maxw]
                si.on_wait = keep
                for i in range(0, len(extra), maxw):
                    nop = mybir.InstNoOp(
                        name=f"{inst.name}-sw{i}", ins=[], outs=[]
                    )
                    nop.engine = inst.engine
                    nop.sync_info = mybir.SyncInfo(
                        on_wait=extra[i : i + maxw], on_update=[]
                    )
                    nc.register_instruction(nop)
                    new_list.append(nop)
            new_list.append(inst)
        bb.instructions[:] = new_list


class _Group:
    """Tracks start/stop flags for a PSUM accumulation group whose matmuls
    are emitted in several program-order batches."""

    def __init__(self, total):
        self.total = total
        self.emitted = 0

    def flags(self):
        start = self.emitted == 0
        self.emitted += 1
        return start, self.emitted == self.total


FILL_CQ = 0     # junk matmuls before each layer.s c/q-dependent block
FILL_P1 = 0      # before each phase-1 PE zone-sum chunk group
FILL_RAMP = 0   # bridging the phase-1 -> decode transition


def build_kernel(nsteps=12):
    """Build the per-core Bass graph (SPMD: same graph on all 8 cores)."""
    _install_tile_drain_patch()
    nc = bass.Bass()

    enc = nc.declare_dram_parameter("enc", [PC, NZ, D], F8, isOutput=False)
    wrz0 = nc.declare_dram_parameter("wrz0", [4, 128, 512], BF, isOutput=False)
    win0 = nc.declare_dram_parameter("win0", [2, 128, 256], BF, isOutput=False)
    whn0 = nc.declare_dram_parameter("whn0", [2, 128, 256], BF, isOutput=False)
    wrz1 = nc.declare_dram_parameter("wrz1", [4, 128, 512], BF, isOutput=False)
    win1 = nc.declare_dram_parameter("win1", [2, 128, 256], BF, isOutput=False)
    whn1 = nc.declare_dram_parameter("whn1", [2, 128, 256], BF, isOutput=False)
    wout = nc.declare_dram_parameter("wout", [2, 128, 256], BF, isOutput=False)
    biasT = nc.declare_dram_parameter("biasT", [8, NGRP * 128], BF,
                                      isOutput=False)
    oh8 = nc.declare_dram_parameter("oh8", [8, 512], BF, isOutput=False)
    brows = nc.declare_dram_parameter("brows", [1, NROW * 128], BF,
                                      isOutput=False)
    out = nc.declare_dram_parameter("out", [PC, STEPS, NZ], BF, isOutput=True)

    with tile.TileContext(nc) as tc:
        with (
            tc.tile_pool(name="consts", bufs=1) as consts,
            tc.tile_pool(name="state", bufs=1) as state,
            tc.tile_pool(name="enc_pool", bufs=4) as enc_pool,
            tc.tile_pool(name="gates", bufs=2) as gates,
            tc.tile_pool(name="ostage", bufs=2) as ostage,
            tc.tile_pool(name="psum", bufs=1, space="PSUM") as psum,
        ):
            # ---- phase 1 DMA: enc all on the sync queue, in tree order ----
            # PE consumes ~2x faster than the DVE tree: it owns the first
            # chunks plus the tail; the DVE takes two early-arriving chunks
            ZCHS = [32, 32, 32, 32, 32, 32, 32, 16, 16]
            PE_CHUNKS = (0, 2, 4, 5, 6, 7)
            NCH = len(ZCHS)
            # enc split across both HWDGE queues (even->sync, odd->scalar)
            # so the two queues' per-chunk completion overheads overlap and
            # the aggregate stream runs at the DMA bandwidth roofline
            e_tiles = []
            z0 = 0
            for i, zch in enumerate(ZCHS):
                e_sb = enc_pool.tile([128, 32 * D], F8, tag="echunk",
                                     bufs=13)
                eng = nc.sync if i % 2 == 0 else nc.scalar
                eng.dma_start(e_sb[:, : zch * D], enc[:, z0 : z0 + zch, :])
                e_tiles.append(e_sb)
                z0 += zch

            # small constants then weights behind the enc stream, ordered
            # by first use
            biasT_sb = consts.tile([8, NGRP * 128], BF, tag="biasT")
            nc.scalar.dma_start(biasT_sb[:], biasT[:])
            oh8_sb = consts.tile([8, 512], BF, tag="oh8")
            nc.scalar.dma_start(oh8_sb[:], oh8[:])
            brow_sb = consts.tile([1, NROW * 128], BF, tag="brow")
            nc.scalar.dma_start(brow_sb[:], brows[:])
            w_sb = {}
            for name, ap, kc, mdim in (
                ("wrz0", wrz0, 4, 512),
                ("whn0", whn0, 2, 256),
                ("wrz1", wrz1, 4, 512),
                ("whn1", whn1, 2, 256),
                ("win1", win1, 2, 256),
                ("wout", wout, 2, 256),
                ("win0", win0, 2, 256),
            ):
                t_ = consts.tile([128, kc, mdim], BF, tag=name)
                nc.scalar.dma_start(t_[:], ap.rearrange("k p m -> p k m"))
                w_sb[name] = t_

            ones_row = consts.tile([1, 128], BF, tag="ones")
            nc.gpsimd.memset(ones_row[:], 1.0)
            identity = consts.tile([128, 128], F32, tag="ident")
            nc.gpsimd.memset(identity[:], 0.0)
            nc.gpsimd.affine_select(
                out=identity[:],
                in_=identity[:],
                compare_op=OP.not_equal,
                fill=1.0,
                base=0,
                pattern=[[-1, 128]],
                channel_multiplier=1,
            )
            ident_f8 = consts.tile([128, 128], F8, tag="identf8")
            nc.gpsimd.memset(ident_f8[:], 0.0)
            nc.gpsimd.affine_select(
                out=ident_f8[:],
                in_=ident_f8[:],
                compare_op=OP.not_equal,
                fill=1.0,
                base=0,
                pattern=[[-1, 128]],
                channel_multiplier=1,
            )
            # prewarm the sigmoid/tanh ACT table during phase 1
            warm = consts.tile([128, 1], F32, tag="warm")
            nc.scalar.activation(warm[:], identity[:, 0:1], AF.Sigmoid)
            gwarm = consts.tile([128, 128], BF, tag="gwarm")
            nc.gpsimd.tensor_tensor(gwarm[:], ident_f8[:], ident_f8[:],
                                    OP.add)

            # PE keepalive: junk matmuls emitted at known stall points keep
            # the tensor engine's p-state at max (idle gaps downclock it and
            # the next real matmuls run 2-4x slower).
            jk_ps = psum.tile([128, 512], F32, tag="junk", bufs=1)

            def fill(n):
                for _ in range(n):
                    nc.tensor.matmul(jk_ps[:, 0:64], gwarm[:],
                                     gwarm[:, 0:64], start=True, stop=True)

            def fill_t(tsrc, n):
                # junk matmuls reading a chain tile: become ready when the
                # chain op lands, bridging PE idle before the next real
                # matmul group (p-state keepalive)
                for _ in range(n):
                    nc.tensor.matmul(jk_ps[:, 0:256], gwarm[:],
                                     tsrc[:, 0:256], start=True, stop=True)

            def fill_e(e_sb, n):
                # junk matmuls whose moving operand is a just-arrived enc
                # chunk: they become ready exactly when the chunk lands,
                # bridging PE idle between chunk arrivals (p-state keepalive)
                for _ in range(n):
                    nc.tensor.matmul(jk_ps[:], ident_f8[:],
                                     e_sb[:, 0:512], start=True, stop=True)

            # ---- phase 1: zone-mean; DVE pairwise tree for most chunks,
            # PE identity-matmul accumulation for PE_CHUNKS (the PE is
            # otherwise idle during the stream; PSUM accumulates in f32)
            tmpf = state.tile([128, 512], F32, tag="tmpf")
            ptl = state.tile([128, 256], F32, tag="ptl")
            acc = state.tile([128, 256], F32, tag="acc")
            zsum = psum.tile([128, 256], F32, tag="outp", bufs=2)
            n_pe = sum(ZCHS[i] for i in PE_CHUNKS)
            pe_grp = _Group(n_pe)
            first_dve = True
            for i in range(NCH):
                e_sb = e_tiles[i]
                if i in PE_CHUNKS:
                    for z in range(ZCHS[i]):
                        st, sp = pe_grp.flags()
                        nc.tensor.matmul(
                            zsum[:], ident_f8[:],
                            e_sb[:, z * D : (z + 1) * D],
                            start=st, stop=sp,
                        )
                    continue
                w = ZCHS[i] * D
                scr = state.tile([128, 16 * D], BF, tag="scr")
                h = w // 2
                nc.vector.tensor_tensor(
                    scr[:, 0:h], e_sb[:, 0:h], e_sb[:, h:w], OP.add
                )
                w = h
                while w > 4 * D:
                    h = w // 2
                    nc.vector.tensor_tensor(
                        scr[:, 0:h], scr[:, 0:h], scr[:, h:w], OP.add
                    )
                    w = h
                nc.vector.tensor_tensor(
                    tmpf[:], scr[:, 0 : 2 * D], scr[:, 2 * D : 4 * D], OP.add
                )
                if first_dve:
                    nc.vector.tensor_tensor(
                        acc[:], tmpf[:, 0:D], tmpf[:, D : 2 * D], OP.add
                    )
                    first_dve = False
                else:
                    nc.vector.tensor_tensor(
                        ptl[:], tmpf[:, 0:D], tmpf[:, D : 2 * D], OP.add
                    )
                    nc.vector.tensor_tensor(acc[:], acc[:], ptl[:], OP.add)
            ztot = state.tile([128, 256], F32, tag="ztot")
            nc.scalar.activation(ztot[:], zsum[:], AF.Copy)

            # ---- state: actT slots [h0c0, h0c1, h1c0, h1c1] ----
            # the acc + ztot merge rides the PSUM accumulation of the
            # two transposes
            actT = state.tile([128, 4, 128], BF, tag="actT")
            for c in range(2):
                cs = slice(c * 128, (c + 1) * 128)
                ctps = psum.tile([128, 128], F32, tag="outp", bufs=2)
                nc.tensor.matmul(ctps[:], acc[:, cs], identity[:],
                                 is_transpose=True, start=True, stop=False)
                nc.tensor.matmul(ctps[:], ztot[:, cs], identity[:],
                                 is_transpose=True, start=False, stop=True)
                nc.scalar.activation(actT[:, c, :], ctps[:], AF.Copy,
                                     scale=1.0 / NZ)
                nc.scalar.activation(actT[:, 2 + c, :], ctps[:], AF.Copy,
                                     scale=1.0 / NZ)

            # ---- decode-phase emitters ----
            def gate_mms(g, grp, w_t, kis, slots, mlo, mhi, coloff=0):
                """slots entries: int -> actT slot; (tile, k) -> gates tile
                chunk k used as the moving operand."""
                for m in range(mlo, mhi):
                    ms = slice((coloff + m) * 128, (coloff + m + 1) * 128)
                    wms = slice(m * 128, (m + 1) * 128)
                    for ki, slot in zip(kis, slots):
                        if isinstance(slot, tuple):
                            src, k = slot
                            rhs = src[:, k * 128 : (k + 1) * 128]
                        else:
                            rhs = actT[:, slot, :]
                        st, sp = grp.flags()
                        nc.tensor.matmul(
                            g[:, ms], w_t[:, ki, wms], rhs, start=st, stop=sp,
                        )

            # ---- phase 2: 12 decode steps ----
            cur = {}

            def bias_mm(g, grp, gidx):
                """Start a bank's single accumulation group by adding all
                four 128-chunk biases via one K=8 matmul against a one-hot
                moving operand (rows 0:4 bf16-hi, 4:8 bf16-lo)."""
                st, sp = grp.flags()
                nc.tensor.matmul(
                    g[:, 0:512], biasT_sb[0:8, gidx * 128 : (gidx + 1) * 128],
                    oh8_sb[0:8, 0:512], start=st, stop=sp,
                )

            def emit_pre0(t):
                """rz0-hh / hn0 / biases for step t: depend only on
                h0(t-1)."""
                g_rz0 = psum.tile([128, 512], F32, tag="rz0", bufs=2)
                g_hi0 = psum.tile([128, 512], F32, tag="hi0", bufs=1)
                grz0 = _Group(1 + 8 + (12 if t > 0 else 0))
                ghi0 = _Group(1 + 4 + (4 if t > 0 else 0))
                bias_mm(g_rz0, grz0, BT_RZ0 + t)
                bias_mm(g_hi0, ghi0, BT_HI0 + t)
                gate_mms(g_rz0, grz0, w_sb["wrz0"], (2, 3), (0, 1), 0, 4)
                gate_mms(g_hi0, ghi0, w_sb["whn0"], (0, 1), (0, 1), 0, 2)
                cur[t] = (g_rz0, g_hi0, grz0, ghi0)

            fill(FILL_RAMP)
            emit_pre0(0)
            prev_w = None
            prev_cq = None   # (c_, q_) of the most recent layer-1 chain

            def emit_wout_mms():
                """wout matmuls on h1 (read actT slots 2,3 -- must be
                emitted before layer-1 overwrites them)."""
                g_pb = psum.tile([128, 256], F32, tag="outp", bufs=2)
                gout = _Group(3)
                st, sp = gout.flags()
                nc.tensor.matmul(
                    g_pb[:], ones_row[:],
                    brow_sb[0:1, _BOUT * 128 : (_BOUT + 2) * 128],
                    start=st, stop=sp,
                )
                for ki, slot in ((0, 2), (1, 3)):
                    st, sp = gout.flags()
                    nc.tensor.matmul(
                        g_pb[:], actT[:, slot, :], w_sb["wout"][:, ki, :],
                        start=st, stop=sp,
                    )
                return g_pb

            def emit_wout_copy(g_pb, t):
                """PSUM->SBUF stage + output DMA; emitted at the end of the
                layer-1 block so the ACT COPY queues after the critical
                tanh, not before it."""
                o_ = ostage.tile([128, 256], BF, tag="ost")
                nc.scalar.activation(o_[:], g_pb[:], AF.Copy)
                nc.sync.dma_start(out[:, t, :], o_[:])

            for t in range(nsteps):
                g_rz0, g_hi0, grz0, ghi0 = cur.pop(t)
                g_rz1 = psum.tile([128, 512], F32, tag="rz1", bufs=1)
                g_hi1 = psum.tile([128, 512], F32, tag="hi1", bufs=1)
                grz1 = _Group(1 + 8 + 12)
                ghi1 = _Group(1 + 4 + 4)

                for layer in range(2):
                    if layer == 0:
                        g_rz, g_hi, grz, ghi = g_rz0, g_hi0, grz0, ghi0
                        h_sl, x_sl = 0, (2, 3)      # h slots; x = other h
                        w_f, w_in = w_sb["wrz0"], w_sb["win0"]
                        cq = prev_cq if t > 0 else None
                    else:
                        g_rz, g_hi, grz, ghi = g_rz1, g_hi1, grz1, ghi1
                        h_sl, x_sl = 2, (0, 1)
                        w_f, w_in = w_sb["wrz1"], w_sb["win1"]
                        cq = this_cq  # layer-0 chain of this step

                    # r-part of the input block via W@c + (-W)@q: fires as
                    # soon as the previous chain's c/q exist (before h');
                    # in-gate mms right behind so gin is in PSUM before v_
                    s_ = gates.tile([128, 512], BF, tag=f"s{layer}")
                    hnb = gates.tile([128, 256], BF, tag=f"hb{layer}")
                    if cq is not None:
                        c_p, q_p = cq
                        fill(FILL_CQ)
                        gate_mms(g_rz, grz, w_f, (0, 1),
                                 ((c_p, 0), (c_p, 1)), 0, 2)
                        gate_mms(g_rz, grz, w_f, (0, 1),
                                 ((q_p, 0), (q_p, 1)), 0, 2)
                    # stage the hn gate to SBUF bf16 on the DVE; emitted
                    # BEFORE the in-mms so its (tile-granular) g_hi dep
                    # closes at the long-done hn mms -> runs in DVE idle
                    nc.vector.tensor_scalar_add(hnb[:], g_hi[:, 0:256], 0.0)
                    if cq is not None:
                        gate_mms(g_hi, ghi, w_in, (0, 1), x_sl, 0, 2,
                                 coloff=2)
                    # biases pre-accumulated into PSUM: single 256-wide ACTs
                    nc.scalar.activation(s_[:, 0:256], g_rz[:, 0:256],
                                         AF.Sigmoid)
                    # z-part plainly on h' of the previous chain
                    if cq is not None:
                        gate_mms(g_rz, grz, w_f, (0, 1), x_sl, 2, 4)
                    nc.scalar.activation(s_[:, 256:512], g_rz[:, 256:512],
                                         AF.Sigmoid)
                    t_ = gates.tile([128, 256], BF, tag=f"t{layer}")
                    nc.vector.tensor_tensor(t_[:], hnb[:], s_[:, 0:256],
                                            OP.mult)
                    if layer == 0:
                        # layer-1 parts that need only h1(t-1)
                        bias_mm(g_rz1, grz1, BT_RZ1)
                        bias_mm(g_hi1, ghi1, BT_HI1)
                        gate_mms(g_rz1, grz1, w_sb["wrz1"], (2, 3), (2, 3),
                                 0, 4)
                        gate_mms(g_hi1, ghi1, w_sb["whn1"], (0, 1), (2, 3),
                                 0, 2)
                    else:
                        # step t+1 parts that need only h0'(t)
                        if t + 1 < nsteps:
                            emit_pre0(t + 1)
                    v_ = gates.tile([128, 256], BF, tag=f"v{layer}")
                    nc.vector.scalar_tensor_tensor(
                        v_[:], g_hi[:, 256:512], 0.0, t_[:],
                        op0=OP.add, op1=OP.add,
                    )
                    # 256-wide tail: n/c/zm1/q/h'.  q = (z-1)*n as a 2x TT
                    # against the precomputed zm1 (the STT form runs 1x).
                    c_ = gates.tile([128, 256], BF, tag=f"c{layer}")
                    n_ = gates.tile([128, 256], BF, tag=f"n{layer}")
                    q_ = gates.tile([128, 256], BF, tag=f"q{layer}")
                    zm1 = gates.tile([128, 256], BF, tag=f"z{layer}")
                    nc.scalar.activation(n_[:], v_[:], AF.Tanh)
                    nc.vector.tensor_tensor(
                        c_[:], s_[:, 256:512], actT[:, h_sl : h_sl + 2, :],
                        OP.mult,
                    )
                    nc.vector.tensor_scalar(zm1[:], s_[:, 256:512], 1.0,
                                            -1.0, op0=OP.subtract,
                                            op1=OP.mult)
                    nc.vector.tensor_tensor(q_[:], zm1[:], n_[:], OP.mult)
                    nc.vector.tensor_tensor(
                        actT[:, h_sl : h_sl + 2, :], c_[:], q_[:],
                        OP.add,
                    )

                    if layer == 0:
                        this_cq = (c_, q_)
                        if prev_w is not None:
                            prev_pb = emit_wout_mms()
                    else:
                        prev_cq = (c_, q_)
                        if prev_w is not None:
                            emit_wout_copy(prev_pb, prev_w)
                prev_w = t
            g_pb = emit_wout_mms()
            emit_wout_copy(g_pb, prev_w)

    _split_waits(nc)
    return nc


def _prep_inputs(encoded_features, step_emb, W_ih0, W_hh0, b_ih0, b_hh0,
                 W_ih1, W_hh1, b_ih1, b_hh1, W_out, b_out):
    """Host-side: slice/shard the big input, transpose + cast weights,
    fold the output projection into layer-0 input weights, fold the
    step-embedding matmul + all additive constants into bias columns."""
    f4 = np.float32
    enc_last = np.asarray(encoded_features)[:, -1].astype(ml_dtypes.float8_e4m3)
    enc_last = np.ascontiguousarray(enc_last)

    W_ih0 = np.asarray(W_ih0, f4)
    W_hh0 = np.asarray(W_hh0, f4)
    W_ih1 = np.asarray(W_ih1, f4)
    W_hh1 = np.asarray(W_hh1, f4)
    W_out = np.asarray(W_out, f4)
    step_emb = np.asarray(step_emb, f4)
    b_ih0 = np.asarray(b_ih0, f4)
    b_hh0 = np.asarray(b_hh0, f4)
    b_ih1 = np.asarray(b_ih1, f4)
    b_hh1 = np.asarray(b_hh1, f4)
    b_out = np.asarray(b_out, f4)

    W_emb = W_ih0[:, :D]          # (768, 256)
    W_pred = W_ih0[:, D:]         # (768, 256)
    W_fold = W_pred @ W_out       # (768, 256): pred feedback folded onto h1
    b_fold = W_pred @ b_out       # (768,)

    # gi_emb[t] = W_emb @ step_emb[t] + b_ih0  -> (12, 768)
    gi_emb = step_emb[:STEPS] @ W_emb.T + b_ih0[None, :]

    def kstack(*mats_cols):
        chunks = []
        for mat, cols in mats_cols:
            mt = np.ascontiguousarray(mat.T[:, cols])  # (K, M)
            for k in range(0, mt.shape[0], 128):
                chunks.append(mt[k : k + 128])
        return np.stack(chunks).astype(BF16)  # (nk, 128, M)

    rz = slice(0, 512)
    ng = slice(512, 768)
    wrz0 = kstack((W_fold, rz), (W_hh0, rz))          # K: h1c0,h1c1,h0c0,h0c1
    win0 = kstack((W_fold, ng))
    whn0 = kstack((W_hh0, ng))
    wrz1 = kstack((W_ih1, rz), (W_hh1, rz))           # K: h0c0,h0c1,h1c0,h1c1
    win1 = kstack((W_ih1, ng))
    whn1 = kstack((W_hh1, ng))
    wout = np.stack([np.ascontiguousarray(W_out.T)[k : k + 128] for k in (0, 128)]
                    ).astype(BF16)                    # (2, 128, 256)

    biasT = np.zeros((8, NGRP * 128), f4)

    def putg(g, vec):
        # bf16 hi/lo split: rows 0:nk hold bf16(vec) chunks, nk:2nk residual
        nk = len(vec) // 128
        hi = vec.astype(BF16).astype(f4)
        lo = vec - hi
        for k in range(nk):
            biasT[k, g * 128 : (g + 1) * 128] = hi[k * 128 : (k + 1) * 128]
            biasT[nk + k, g * 128 : (g + 1) * 128] = (
                lo[k * 128 : (k + 1) * 128])

    for t in range(STEPS):
        extra = b_fold if t > 0 else np.zeros_like(b_fold)
        putg(BT_RZ0 + t, gi_emb[t, :512] + b_hh0[:512] + extra[:512])
        putg(BT_HI0 + t, np.concatenate(
            [b_hh0[512:], gi_emb[t, 512:] + extra[512:]]))
    putg(BT_RZ1, b_ih1[:512] + b_hh1[:512])
    putg(BT_HI1, np.concatenate([b_hh1[512:], b_ih1[512:]]))
    biasT = biasT.astype(BF16)

    oh8 = np.zeros((8, 512), f4)
    for k in range(8):
        oh8[k, (k % 4) * 128 : (k % 4 + 1) * 128] = 1.0
    oh8 = oh8.astype(BF16)

    brows = np.zeros(NROW * 128, f4)
    brows[_BOUT * 128 : _BOUT * 128 + 256] = b_out
    brows = brows.astype(BF16)[None, :]

    shared = dict(wrz0=wrz0, win0=win0, whn0=whn0, wrz1=wrz1,
                  win1=win1, whn1=whn1, wout=wout,
                  biasT=biasT, oh8=oh8, brows=brows)
    in_maps = []
    for i in range(N_CORES):
        m = dict(shared)
        m["enc"] = enc_last[i * PC : (i + 1) * PC]
        in_maps.append(m)
    return in_maps


_CACHE = {}


def _run(in_maps, trace=False):
    from concourse.bass_utils import run_bass_kernel_spmd

    if "nc" not in _CACHE:
        _CACHE["nc"] = build_kernel()
    nc = _CACHE["nc"]
    res = run_bass_kernel_spmd(
        nc, in_maps, core_ids=list(range(N_CORES)), trace=trace
    )
    preds = np.concatenate([res.results[i]["out"] for i in range(N_CORES)],
                       axis=0).astype(np.float32)
    return preds, res


def kernel(encoded_features, step_emb, W_ih0, W_hh0, b_ih0, b_hh0,
           W_ih1, W_hh1, b_ih1, b_hh1, W_out, b_out, num_steps):
    assert int(num_steps) == STEPS
    in_maps = _prep_inputs(encoded_features, step_emb, W_ih0, W_hh0, b_ih0,
                           b_hh0, W_ih1, W_hh1, b_ih1, b_hh1, W_out, b_out)
    preds, _ = _run(in_maps, trace=False)
    return preds

